# revision 36
# baseline (speedup 1.0000x reference)
"""Trainium2 Bass kernel for nn_LDS_LR: low-rank LDS + AR low-rank correction.

Math (per batch b):
    Bu   = X @ B1 @ B2                      # [T, N] rank-64 input projection
    h_t  = A * h_{t-1} + Bu_t               # diagonal recurrence, h_{-1} = h0
    lds  = H @ C1 @ C2                      # [T, O] rank-64 output projection
    proj = einsum('ti,rik->trk', X, M1)     # [T, R, KX]
    ar_t = sum_k M2[:,:,k] @ proj[t-k,:,k]  # AR with KX=5 taps
    Y    = lds + ar

Sharding: 8 cores = 4 batches x 2 sequence halves (1024 steps each).

v2 design notes (vs the v1 carry-scan kernel):
  * The chunk-boundary carry h_1023 = sum_s A^(1023-s) Bu_prev[s] + A^1024 h0
    is computed WITHOUT prefix scans: the decay-power matrix
    APn[n,s] = A[n]^(1023-s) is precomputed host-side (weights-only
    transform), and one fused multiply+reduce (tensor_tensor_reduce on DVE /
    scalar_tensor_tensor+accum on Pool) per (n-tile, t-block) yields the
    carry.  This removes 8 full-length scans (~18.6us of DVE).
  * All matmul loops are k-outer so consecutive matmuls share a stationary
    operand (LDWEIGHTS serializes against same-row-group matmuls; halving
    loads keeps the PE column-issue cadence near peak).  PE emission is
    software-pipelined so the PE never waits on DVE scans (avoids HAM
    re-throttle windows triggered by PE idle gaps).
  * Own-half scans run per 512-col block, block-0 first, so CH1/Y/output-DMA
    of block 0 overlap block-1 scans.  A few block-1 scans run on the Pool
    engine in parallel with DVE.
  * All DRAM operands are host-packed into [128, *] row-major blobs so each
    loads with 1-2 large DMAs (11 input DMAs total, on the two hardware DGE
    queues: SP and Activation).  Output is bf16 (host casts back to fp32).
"""

import contextlib
import ctypes
import os
import sys
import types

import numpy as np
from contextlib import ExitStack

import concourse.bass as bass
import concourse.tile as tile
from concourse import bacc, mybir
from concourse.bass_utils import run_bass_kernel_spmd


def _install_ntff_hook():
    """Provide antenv.axon_hooks.get_axon_ntff_profile_hook if the image
    lacks it, driving NTFF capture via the libaxon_pjrt C ABI directly."""
    try:
        from antenv.axon_hooks import get_axon_ntff_profile_hook  # noqa: F401
        return
    except ImportError:
        pass
    so_path = "/opt/axon/libaxon_pjrt.so"
    hook = None
    if os.path.exists(so_path):
        lib = ctypes.CDLL(so_path)
        if hasattr(lib, "axon_start_nrt_profile"):
            lib.axon_start_nrt_profile.argtypes = [
                ctypes.POINTER(ctypes.c_int64), ctypes.c_size_t]
            lib.axon_start_nrt_profile.restype = ctypes.c_int64
            lib.axon_stop_nrt_profile.argtypes = [ctypes.c_char_p]
            lib.axon_stop_nrt_profile.restype = ctypes.c_int64

            @contextlib.contextmanager
            def _hook(output_dir, device_ids):
                import jax
                jax.devices()
                if device_ids:
                    ids = (ctypes.c_int64 * len(device_ids))(*device_ids)
                    rc = lib.axon_start_nrt_profile(ids, len(device_ids))
                else:
                    rc = lib.axon_start_nrt_profile(None, 0)
                if rc != 0:
                    raise RuntimeError(f"axon_start_nrt_profile rc={rc}")
                try:
                    yield
                finally:
                    n = lib.axon_stop_nrt_profile(str(output_dir).encode())
                    print(f"ntff profile: {n} file(s) -> {output_dir}",
                          file=sys.stderr)

            hook = _hook
    mod = types.ModuleType("antenv.axon_hooks")
    mod.get_axon_ntff_profile_hook = lambda: hook
    mod.set_axon_ntff_profile_hook = lambda h: None
    sys.modules["antenv.axon_hooks"] = mod


_install_ntff_hook()

DT = mybir.dt.float32
_MDT_NAME = os.environ.get("KERNEL_MDT", "bf16")
MDT = {"f32": mybir.dt.float32, "f32r": mybir.dt.float32r,
       "bf16": mybir.dt.bfloat16}[_MDT_NAME]
MNP = mybir.dt.np(MDT)
F32 = np.float32

OUT_BF16 = bool(int(os.environ.get("KERNEL_OUT_BF16", "1")))
# "1": stride-0 broadcast AP; "act": materialize A rows on the scalar engine;
# "0": materialize on the Pool engine (slow, debug only)
BCAST = os.environ.get("KERNEL_BCAST", "1")
# Pool engine cannot touch PSUM (BIR verifier), so offloading b1 scans to it
# requires a scalar-engine PSUM->SBUF staging copy per tile first.
GPS_SCAN = int(os.environ.get("KERNEL_GPS_SCAN", "0"))
USE_TTR = bool(int(os.environ.get("KERNEL_TTR", "1")))   # fused reduce for carry
SPLIT_SCAN = bool(int(os.environ.get("KERNEL_SPLIT_SCAN", "1")))
# ttr (tensor_tensor_reduce) FAULTS trn2 hardware (NRT_EXEC_UNIT_UNRECOVERABLE)
# despite passing CoreSim.  "pe" computes the carry with matmuls (G^T via
# identity-matmul, V = G^T @ AP2, then V*B2 summed by a ones-matmul), freeing
# the DVE for the own-half scans; "stt"/"scan" are DVE fallbacks.
CARRY = os.environ.get("KERNEL_CARRY", "pe")  # pe | ttr | stt | scan
INTERLEAVE = bool(int(os.environ.get("KERNEL_INTERLEAVE", "1")))
SCALQ = bool(int(os.environ.get("KERNEL_SCALQ", "1")))  # scalar-engine DMA queue
VECCOPY = bool(int(os.environ.get("KERNEL_VECCOPY", "1")))

ODT = MDT if OUT_BF16 else DT
ONP = mybir.dt.np(ODT)

B, T, D = 4, 2048, 1024
NST, R, KX, OUT = 1024, 64, 5, 1024
TC = 1024          # per-core chunk length
TBL = 512          # time block (one PSUM bank at fp32)

_CACHED_NC = None
LAST_RESULT = None  # BassKernelResults of the most recent run (for test.py)

MULT = mybir.AluOpType.mult
ADD = mybir.AluOpType.add


def _emit(ctx, tc, io):
    nc = tc.nc
    xo, xp, w1, b2, c1, w2, ap, avio, ident, yt = io

    wp = ctx.enter_context(tc.tile_pool(name="wp", bufs=1))
    xpool = ctx.enter_context(tc.tile_pool(name="xpool", bufs=1))
    hp = ctx.enter_context(tc.tile_pool(name="hp", bufs=1))
    pp = ctx.enter_context(tc.tile_pool(name="pp", bufs=1))
    yp = ctx.enter_context(tc.tile_pool(name="yp", bufs=1))
    # PSUM: pA (j/Y) 2 banks, pB (Bu) 4 banks, pC (G/CH1/tails) 2 banks = 8
    pA = ctx.enter_context(tc.tile_pool(name="pA", bufs=2, space="PSUM"))
    pB = ctx.enter_context(
        tc.tile_pool(name="pB", bufs=(4 if SPLIT_SCAN else 2), space="PSUM"))
    pC = ctx.enter_context(tc.tile_pool(name="pC", bufs=2, space="PSUM"))

    # ---------------- input DMAs (three queues, deadline-ordered) -----------
    # sync (Q1, earliest start): xo quarters, b2, AP halves  (+ y outs later)
    # gpsimd (Q0): w1g, w1r, xp quarters, ident
    # scalar (Q10, latest start): c1, w2, avio
    q2 = nc.scalar if SCALQ else nc.sync
    xots = [xpool.tile([128, 2048], MDT, tag=f"xoq{i}", name=f"xoq{i}")
            for i in range(4)]
    for i in range(4):
        nc.sync.dma_start(xots[i][:], xo[:, i * 2048:(i + 1) * 2048])
    b2sb = wp.tile([64, 1024], MDT, tag="b2", name="b2sb")
    nc.sync.dma_start(b2sb[:], b2[:])
    apa = xpool.tile([128, 4096], MDT, tag="apa", name="apa")
    apb = xpool.tile([128, 4096], MDT, tag="apb", name="apb")
    nc.sync.dma_start(apa[:], ap[:, 0:4096])
    nc.sync.dma_start(apb[:], ap[:, 4096:8192])

    w1gsb = wp.tile([128, 1024], MDT, tag="w1g", name="w1gsb")
    nc.gpsimd.dma_start(w1gsb[:], w1[:, 0:1024])
    w1rsb = wp.tile([128, 2048], MDT, tag="w1r", name="w1rsb")
    nc.gpsimd.dma_start(w1rsb[:], w1[:, 1024:3072])
    xpts = [xpool.tile([128, 2048], MDT, tag=f"xpq{i}", name=f"xpq{i}")
            for i in range(4)]
    for i in range(4):
        nc.gpsimd.dma_start(xpts[i][:], xp[:, i * 2048:(i + 1) * 2048])
    identsb = wp.tile([64, 65], MDT, tag="ident", name="identsb")
    nc.gpsimd.dma_start(identsb[:], ident[:])

    c1sb = wp.tile([128, 512], MDT, tag="c1", name="c1sb")
    q2.dma_start(c1sb[:], c1[:])
    w2sb = wp.tile([128, 3072], MDT, tag="w2", name="w2sb")
    q2.dma_start(w2sb[:], w2[:])
    aviosb = wp.tile([128, 16], DT, tag="avio", name="aviosb")
    q2.dma_start(aviosb[:], avio[:])

    def xpt(k):  # xp k-tile [128, 1024]
        return xpts[k // 2][:, (k % 2) * 1024:(k % 2 + 1) * 1024]

    def xot(k):
        return xots[k // 2][:, (k % 2) * 1024:(k % 2 + 1) * 1024]

    def apt(n, tb):  # AP slice for (n-tile, t-block) [128, 512]
        t = apa if n < 4 else apb
        return t[:, (n % 4) * 1024 + tb * 512:(n % 4) * 1024 + (tb + 1) * 512]

    def w1t(k, lo, hi):  # W1 k-tile column slice (w1g: cols 0:128, w1r: rest)
        if hi <= 128:
            return w1gsb[:, k * 128 + lo:k * 128 + hi]
        assert lo >= 128
        return w1rsb[:, k * 256 + lo - 128:k * 256 + hi - 128]

    def w2t(m, o):  # W2 stationary for (m-tile, o-tile) [128, 128]
        return w2sb[:, m * 1024 + o * 128:m * 1024 + (o + 1) * 128]

    # A-broadcast for the scans: stride-0 view of avio column n (fallback:
    # materialized [128, TC] tiles on the Pool engine).
    if BCAST == "1":
        def abv(n, tb):
            return aviosb[:, n:n + 1].broadcast_to((128, 512))

        def abv_full(n):
            return aviosb[:, n:n + 1].broadcast_to((128, TC))
    else:
        ABW = 512 if (SPLIT_SCAN and CARRY != "scan") else TC
        ones = wp.tile([128, ABW], DT, tag="ones", name="ones")
        absb = []
        if BCAST == "act":
            nc.vector.memset(ones[:], 1.0)
            for n in range(8):
                ab = wp.tile([128, ABW], DT, tag=f"ab{n}", name=f"ab{n}")
                nc.scalar.mul(ab[:], ones[:], aviosb[:, n:n + 1])
                absb.append(ab)
        else:
            nc.gpsimd.memset(ones[:], 1.0)
            for n in range(8):
                ab = wp.tile([128, ABW], DT, tag=f"ab{n}", name=f"ab{n}")
                nc.gpsimd.tensor_scalar_mul(ab[:], ones[:], aviosb[:, n:n + 1])
                absb.append(ab)

        def abv(n, tb):
            return absb[n][:, 0:512]

        def abv_full(n):
            return absb[n][:]

    # ---------------- G_prev = (X_prev @ B1)^T and P_ext j0 ------------------
    gsb = wp.tile([64, 1024], MDT, tag="gprev", name="gsb")
    g_ps = [pC.tile([64, TBL], DT, tag="pc", name=f"g_ps{t}") for t in range(2)]
    PW = 4 + TC + 4
    pext = [pp.tile([128, PW], MDT, tag=f"pext{j}", name=f"pext{j}")
            for j in range(3)]
    j_ps = [pA.tile([128, TBL], DT, tag="pa", name=f"j0_ps{t}")
            for t in range(2)]

    def mm_g(k):
        for t in range(2):
            nc.tensor.matmul(g_ps[t][:], w1t(k, 0, 64),
                             xpt(k)[:, t * TBL:(t + 1) * TBL],
                             start=(k == 0), stop=(k == 7))

    def mm_j0(k):
        for t in range(2):
            nc.tensor.matmul(j_ps[t][:], w1t(k, 0, 128),
                             xot(k)[:, t * TBL:(t + 1) * TBL],
                             start=(k == 0), stop=(k == 7))

    if CARRY == "pe":
        # own-half j0 first (xo + w1g arrive first); weave G into its tail
        for k in range(4):
            mm_j0(k)
        for k in range(4, 8):
            mm_j0(k)
            mm_g(k - 4)
        for k in range(4, 8):
            mm_g(k)
    elif INTERLEAVE:
        for k in range(8):
            mm_g(k)
        for k in range(8):
            mm_j0(k)
    else:
        for t in range(2):
            for k in range(8):
                nc.tensor.matmul(g_ps[t][:], w1t(k, 0, 64),
                                 xpt(k)[:, t * TBL:(t + 1) * TBL],
                                 start=(k == 0), stop=(k == 7))
        for t in range(2):
            for k in range(8):
                nc.tensor.matmul(j_ps[t][:], w1t(k, 0, 128),
                                 xot(k)[:, t * TBL:(t + 1) * TBL],
                                 start=(k == 0), stop=(k == 7))
    for t in range(2):
        nc.scalar.copy(pext[0][:, 4 + t * TBL:4 + (t + 1) * TBL], j_ps[t][:])
    for t in range(2):
        nc.scalar.copy(gsb[:, t * TBL:(t + 1) * TBL], g_ps[t][:])

    # ---------------- carry: Bu_prev + fused AP-weighted reduce --------------
    # iown[n] = sum_s A^(1023-s) Bu_prev[n,s] + ioff[n]; ioff host-folds
    # A^1024 h0 (half-1) / h0 (half-0, xp=0 so the sum vanishes).
    iown = [wp.tile([128, 1], DT, tag=f"iown{n}", name=f"iown{n}")
            for n in range(8)]
    tmpv = wp.tile([128, 512], MDT, tag="tmpv", name="tmpv")
    gacc = [wp.tile([128, 1], DT, tag=f"gacc{i}", name=f"gacc{i}")
            for i in range(1)]
    cstmp = (wp.tile([128, TC], DT, tag="cstmp", name="cstmp")
             if CARRY == "scan" else None)

    def emit_bup(n):
        if SPLIT_SCAN:
            bu = [pB.tile([128, TBL], DT, tag="pb", name=f"bup{n}_{t}")
                  for t in range(2)]
        else:
            bun = pB.tile([128, TC], DT, tag="pb", name=f"bup{n}")
            bu = [bun[:, 0:TBL], bun[:, TBL:TC]]
        for t in range(2):
            nc.tensor.matmul(bu[t][:], b2sb[:, n * 128:(n + 1) * 128],
                             gsb[:, t * TBL:(t + 1) * TBL],
                             start=True, stop=True)
        ioffap = aviosb[:, 8 + n:9 + n]
        if CARRY == "ttr":
            # DVE: fused (bu*AP) elementwise + chained reduce, initial=ioff
            nc.vector.tensor_tensor_reduce(
                tmpv[:], bu[0][:], apt(n, 0), 1.0, ioffap, MULT, ADD,
                accum_out=gacc[0][:])
            nc.vector.tensor_tensor_reduce(
                tmpv[:], bu[1][:], apt(n, 1), 1.0, gacc[0][:], MULT, ADD,
                accum_out=iown[n][:])
        elif CARRY == "stt":
            # stt mult + plain reduce + adds
            XAX = mybir.AxisListType.X
            nc.vector.scalar_tensor_tensor(
                tmpv[:], bu[0][:], 1.0, apt(n, 0), MULT, MULT)
            nc.vector.tensor_reduce(gacc[0][:], tmpv[:], XAX, ADD)
            nc.vector.scalar_tensor_tensor(
                tmpv[:], bu[1][:], 1.0, apt(n, 1), MULT, MULT)
            nc.vector.tensor_reduce(iown[n][:], tmpv[:], XAX, ADD)
            nc.vector.tensor_scalar_add(iown[n][:], iown[n][:], gacc[0][:])
            nc.vector.tensor_scalar_add(iown[n][:], iown[n][:], ioffap)
        else:
            # v1-style carry scans (fp32 temp), iown = cs[-1] + ioff
            nc.vector.tensor_tensor_scan(
                cstmp[:, 0:TBL], abv(n, 0), bu[0][:], 0.0, MULT, ADD)
            nc.vector.tensor_tensor_scan(
                cstmp[:, TBL:TC], abv(n, 1), bu[1][:],
                cstmp[:, TBL - 1:TBL], MULT, ADD)
            nc.vector.tensor_scalar_add(iown[n][:], cstmp[:, TC - 1:TC],
                                        ioffap)

    if CARRY == "pe":
        # -------- carry on the PE: G^T tiles, V = G^T @ AP2, reduce ---------
        # gt[st] = (G[:, st*128:(st+1)*128])^T  via identity matmul
        gtsb = [wp.tile([128, 64], MDT, tag=f"gt{st}", name=f"gt{st}")
                for st in range(8)]
        for st in range(8):
            gt_ps = pA.tile([128, 64], DT, tag="pa", name=f"gt_ps{st}")
            nc.tensor.matmul(gt_ps[:], gsb[:, st * 128:(st + 1) * 128],
                             identsb[0:64, 0:64], start=True, stop=True)
            nc.scalar.copy(gtsb[st][:], gt_ps[:])
        # V[r, n] = sum_s G[r, s] * A[n]^(1023-s)
        v_ps = [pC.tile([64, TBL], DT, tag="pc", name=f"v_ps{nh}")
                for nh in range(2)]
        for st in range(8):
            for nh in range(2):
                nc.tensor.matmul(v_ps[nh][:], gtsb[st][:], apt(st, nh),
                                 start=(st == 0), stop=(st == 7))
        # E = V * B2 (DVE), then iown[n-tile] = ones-matmul over r + ioff
        esb = wp.tile([64, 1024], MDT, tag="esb", name="esb")
        for nh in range(2):
            nc.vector.scalar_tensor_tensor(
                esb[:, nh * TBL:(nh + 1) * TBL], v_ps[nh][:], 1.0,
                b2sb[:, nh * TBL:(nh + 1) * TBL], MULT, MULT)
        iown_ps = pB.tile([128, 8], DT, tag="pb", name="iown_ps")

        def emit_iown_mm():
            for n in range(8):
                nc.tensor.matmul(iown_ps[:, n:n + 1],
                                 esb[:, n * 128:(n + 1) * 128],
                                 identsb[0:64, 64:65], start=True, stop=True)
            for n in range(8):
                nc.vector.tensor_scalar_add(
                    iown[n][:], iown_ps[:, n:n + 1], aviosb[:, 8 + n:9 + n])
    else:
        for n in range(8):
            emit_bup(n)

    # ---------------- own Bu + scans (block-pipelined) -----------------------
    hsb = [hp.tile([128, TC], MDT, tag=f"h{n}", name=f"h{n}") for n in range(8)]

    buo_full = {}

    def emit_buo(n, tb):
        if not SPLIT_SCAN:
            # unsplit fallback: both halves into one [128, TC] tile, then a
            # single full-length scan when tb==1 is requested.
            if tb == 0:
                bun = pB.tile([128, TC], DT, tag="pb", name=f"buo{n}")
                buo_full[n] = bun
                for t in range(2):
                    nc.tensor.matmul(
                        bun[:, t * TBL:(t + 1) * TBL],
                        b2sb[:, n * 128:(n + 1) * 128],
                        pext[0][0:64, 4 + t * TBL:4 + (t + 1) * TBL],
                        start=True, stop=True)
            else:
                nc.vector.tensor_tensor_scan(
                    hsb[n][:], abv_full(n), buo_full[n][:], iown[n][:],
                    MULT, ADD)
            return
        bu = pB.tile([128, TBL], DT, tag="pb", name=f"buo{n}_{tb}")
        nc.tensor.matmul(bu[:], b2sb[:, n * 128:(n + 1) * 128],
                         pext[0][0:64, 4 + tb * TBL:4 + (tb + 1) * TBL],
                         start=True, stop=True)
        if tb == 0:
            nc.vector.tensor_tensor_scan(
                hsb[n][:, 0:TBL], abv(n, 0), bu[:], iown[n][:], MULT, ADD)
        elif n < GPS_SCAN:
            # Pool can't read PSUM: stage bu to SBUF via the scalar engine.
            stg = wp.tile([128, TBL], DT, tag=f"stg{n}", name=f"stg{n}")
            nc.scalar.copy(stg[:], bu[:])
            nc.gpsimd.tensor_tensor_scan(
                hsb[n][:, TBL:TC], abv(n, 1), stg[:],
                hsb[n][:, TBL - 1:TBL], MULT, ADD)
        else:
            nc.vector.tensor_tensor_scan(
                hsb[n][:, TBL:TC], abv(n, 1), bu[:],
                hsb[n][:, TBL - 1:TBL], MULT, ADD)

    # P_ext j1/j2 with the boundary-tail matmuls folded into the same
    # stationary (tap ka -> rows 0:64 shifted by ka, kb -> rows 64:128 by kb).
    def emit_j(j, klo, khi, jps, tl):
        if INTERLEAVE:
            for k in range(klo, khi):
                st = w1t(k, j * 128, (j + 1) * 128)
                for t in range(2):
                    nc.tensor.matmul(jps[t][:], st,
                                     xot(k)[:, t * TBL:(t + 1) * TBL],
                                     start=(k == 0), stop=(k == 7))
                nc.tensor.matmul(tl[:], st, xpt(k)[:, TC - 4:TC],
                                 start=(k == 0), stop=(k == 7))
        else:
            for t in range(2):
                for k in range(klo, khi):
                    nc.tensor.matmul(jps[t][:], w1t(k, j * 128, (j + 1) * 128),
                                     xot(k)[:, t * TBL:(t + 1) * TBL],
                                     start=(k == 0), stop=(k == 7))
            for k in range(klo, khi):
                nc.tensor.matmul(tl[:], w1t(k, j * 128, (j + 1) * 128),
                                 xpt(k)[:, TC - 4:TC],
                                 start=(k == 0), stop=(k == 7))

    def evict_j(j, jps, tl):
        ka, kb = 2 * j - 1, 2 * j
        for t in range(2):
            nc.scalar.copy(
                pext[j][0:64, 4 + ka + t * TBL:4 + ka + (t + 1) * TBL],
                jps[t][0:64, :])
            nc.scalar.copy(
                pext[j][64:128, 4 + kb + t * TBL:4 + kb + (t + 1) * TBL],
                jps[t][64:128, :])
        nc.scalar.copy(pext[j][0:64, 4:4 + ka], tl[0:64, 4 - ka:4])
        nc.scalar.copy(pext[j][64:128, 4:4 + kb], tl[64:128, 4 - kb:4])

    j1ps = [pA.tile([128, TBL], DT, tag="pa", name=f"j1_ps{t}")
            for t in range(2)]
    tl1 = pC.tile([128, 4], DT, tag="pc", name="tl1")
    if CARRY == "pe":
        emit_j(1, 0, 4, j1ps, tl1)
        emit_iown_mm()
        emit_buo(0, 0)
        emit_buo(1, 0)
        emit_j(1, 4, 8, j1ps, tl1)
        evict_j(1, j1ps, tl1)
        emit_buo(2, 0)
        emit_buo(3, 0)
        j2ps = [pA.tile([128, TBL], DT, tag="pa", name=f"j2_ps{t}")
                for t in range(2)]
        tl2 = pC.tile([128, 4], DT, tag="pc", name="tl2")
        emit_j(2, 0, 4, j2ps, tl2)
        emit_buo(4, 0)
        emit_buo(5, 0)
        emit_j(2, 4, 8, j2ps, tl2)
        evict_j(2, j2ps, tl2)
        emit_buo(6, 0)
        emit_buo(7, 0)
    elif INTERLEAVE:
        emit_buo(0, 0)
        emit_buo(1, 0)
        emit_j(1, 0, 4, j1ps, tl1)
        emit_buo(2, 0)
        emit_buo(3, 0)
        emit_j(1, 4, 8, j1ps, tl1)
        evict_j(1, j1ps, tl1)
        emit_buo(4, 0)
        emit_buo(5, 0)
        j2ps = [pA.tile([128, TBL], DT, tag="pa", name=f"j2_ps{t}")
                for t in range(2)]
        tl2 = pC.tile([128, 4], DT, tag="pc", name="tl2")
        emit_j(2, 0, 4, j2ps, tl2)
        emit_buo(6, 0)
        emit_buo(7, 0)
        emit_j(2, 4, 8, j2ps, tl2)
        evict_j(2, j2ps, tl2)
    else:
        emit_j(1, 0, 8, j1ps, tl1)
        evict_j(1, j1ps, tl1)
        j2ps = [pA.tile([128, TBL], DT, tag="pa", name=f"j2_ps{t}")
                for t in range(2)]
        tl2 = pC.tile([128, 4], DT, tag="pc", name="tl2")
        emit_j(2, 0, 8, j2ps, tl2)
        evict_j(2, j2ps, tl2)
        for n in range(8):
            emit_buo(n, 0)

    for n in range(4):
        emit_buo(n, 1)

    # ---------------- CH1 + Y per t-block, streamed out ----------------------
    ysb = [[yp.tile([128, 4 * TBL], ODT, tag=f"y{tb}{g}", name=f"y{tb}{g}")
            for g in range(2)] for tb in range(2)]

    def emit_ch1(tb):
        cps = pC.tile([64, TBL], DT, tag="pc", name=f"c_ps{tb}")
        for n in range(8):
            nc.tensor.matmul(cps[:], c1sb[:, n * 64:(n + 1) * 64],
                             hsb[n][:, tb * TBL:(tb + 1) * TBL],
                             start=(n == 0), stop=(n == 7))
        nc.scalar.copy(pext[0][0:64, 4 + tb * TBL:4 + (tb + 1) * TBL], cps[:])

    def emit_y(tb):
        for o in range(8):
            yps = pA.tile([128, TBL], DT, tag="pa", name=f"y_ps{o}_{tb}")
            for m in range(3):
                nc.tensor.matmul(yps[:], w2t(m, o),
                                 pext[m][:, 4 + tb * TBL:4 + (tb + 1) * TBL],
                                 start=(m == 0), stop=(m == 2))
            g, oo = divmod(o, 4)
            dst = ysb[tb][g][:, oo * TBL:(oo + 1) * TBL]
            if (tb == 0 and o < 6) or not VECCOPY:
                nc.scalar.copy(dst, yps[:])
            else:
                nc.vector.tensor_copy(dst, yps[:])
            if oo == 3:
                nc.sync.dma_start(
                    yt[:, tb * 4096 + g * 2048:tb * 4096 + (g + 1) * 2048],
                    ysb[tb][g][:])

    if SPLIT_SCAN:
        emit_ch1(0)
        for n in range(4, 8):
            emit_buo(n, 1)
    else:
        for n in range(4, 8):
            emit_buo(n, 1)
        emit_ch1(0)
    emit_y(0)
    emit_ch1(1)
    emit_y(1)


def _build():
    nc = bacc.Bacc("TRN2", target_bir_lowering=False, debug=False,
                   num_devices=8)
    xo = nc.dram_tensor("xo", [128, 8192], MDT, kind="ExternalInput").ap()
    xp = nc.dram_tensor("xp", [128, 8192], MDT, kind="ExternalInput").ap()
    w1 = nc.dram_tensor("w1", [128, 3072], MDT, kind="ExternalInput").ap()
    b2 = nc.dram_tensor("b2", [64, 1024], MDT, kind="ExternalInput").ap()
    c1 = nc.dram_tensor("c1", [128, 512], MDT, kind="ExternalInput").ap()
    w2 = nc.dram_tensor("w2", [128, 3072], MDT, kind="ExternalInput").ap()
    ap = nc.dram_tensor("ap", [128, 8192], MDT, kind="ExternalInput").ap()
    avio = nc.dram_tensor("avio", [128, 16], DT, kind="ExternalInput").ap()
    ident = nc.dram_tensor("ident", [64, 65], MDT, kind="ExternalInput").ap()
    yt = nc.dram_tensor("yt", [128, 8192], ODT, kind="ExternalOutput").ap()

    with tile.TileContext(nc) as tc, ExitStack() as ctx:
        _emit(ctx, tc, (xo, xp, w1, b2, c1, w2, ap, avio, ident, yt))
    nc.compile()
    return nc


def _get_nc():
    global _CACHED_NC
    if _CACHED_NC is None:
        _CACHED_NC = _build()
    return _CACHED_NC


def _pack_kt(arr):
    """[1024, C] -> [128, 8*C] with blocks of 128 rows side by side."""
    C = arr.shape[1]
    return np.ascontiguousarray(
        arr.reshape(8, 128, C).transpose(1, 0, 2).reshape(128, 8 * C))


def kernel(inputs, h0, A, B1, B2, C1, C2, M1, M2):
    global LAST_RESULT
    X = np.asarray(inputs, dtype=F32)
    h0 = np.asarray(h0, dtype=F32)
    A64 = np.asarray(A, dtype=np.float64)

    W1 = np.concatenate(
        [np.asarray(B1, dtype=F32)]
        + [np.ascontiguousarray(np.asarray(M1, dtype=F32)[:, :, k].T)
           for k in range(KX)], axis=1)
    W2 = np.concatenate(
        [np.asarray(C2, dtype=F32)]
        + [np.ascontiguousarray(np.asarray(M2, dtype=F32)[:, :, k].T)
           for k in range(KX)], axis=0)
    w1kt = W1.astype(MNP).reshape(8, 128, 384)           # [k, p, c]
    w1p = np.concatenate([                               # [128, 1024 | 2048]
        np.ascontiguousarray(w1kt[:, :, 0:128].transpose(1, 0, 2)
                             .reshape(128, 1024)),
        np.ascontiguousarray(w1kt[:, :, 128:384].transpose(1, 0, 2)
                             .reshape(128, 2048))], axis=1)
    w2p = np.ascontiguousarray(
        W2.reshape(3, 128, 1024).transpose(1, 0, 2)
        .reshape(128, 3072).astype(MNP))                 # [128, 3072]
    b2c = np.ascontiguousarray(np.asarray(B2, dtype=F32).astype(MNP))
    c1p = _pack_kt(np.asarray(C1, dtype=F32).astype(MNP))  # [128, 512]

    # decay powers AP[n, s] = A[n]^(1023-s) (fp64 host-side, cast once)
    powers = (TC - 1) - np.arange(TC, dtype=np.float64)
    APm = np.exp(np.log(A64)[:, None] * powers[None, :])   # [1024, 1024] (n,s)
    if CARRY == "pe":
        # [s-partition, n-free] layout for the V matmuls
        app = _pack_kt(np.ascontiguousarray(APm.T).astype(F32).astype(MNP))
    else:
        app = _pack_kt(APm.astype(F32).astype(MNP))        # [128, 8192]
    identm = np.zeros((64, 65), F32)
    identm[:, 0:64] = np.eye(64, dtype=F32)
    identm[:, 64] = 1.0
    identm = np.ascontiguousarray(identm.astype(MNP))

    ioff_h0 = h0                                           # half 0: plain h0
    ioff_h1 = (A64 ** TC * h0.astype(np.float64)).astype(F32)  # A^1024 h0

    Xbf = X.astype(MNP)
    zeros_xp = np.zeros((128, 8192), MNP)

    def pack_x(b, sl):
        return _pack_kt(np.ascontiguousarray(Xbf[b, sl, :].T))

    in_maps = []
    for c in range(8):
        b, half = divmod(c, 2)
        xoc = pack_x(b, slice(half * TC, (half + 1) * TC))
        if half == 0:
            xpc, ioff = zeros_xp, ioff_h0
        else:
            xpc, ioff = pack_x(b, slice(0, TC)), ioff_h1
        avio = np.zeros((128, 16), F32)
        avio[:, 0:8] = np.asarray(A, dtype=F32).reshape(8, 128).T
        avio[:, 8:16] = ioff.reshape(8, 128).T
        in_maps.append({"xo": xoc, "xp": xpc, "w1": w1p, "b2": b2c,
                        "c1": c1p, "w2": w2p, "ap": app, "avio": avio,
                        "ident": identm})

    nc = _get_nc()
    trace = bool(int(os.environ.get("KERNEL_TRACE", "0")))
    LAST_RESULT = run_bass_kernel_spmd(nc, in_maps, core_ids=list(range(8)),
                                       trace=trace)
    Y = np.empty((B, T, OUT), F32)
    for c in range(8):
        b, half = divmod(c, 2)
        ytc = np.asarray(LAST_RESULT.results[c]["yt"], dtype=F32)
        # yt[p, tb*4096 + o*512 + t] -> Y_core[o*128+p, tb*512+t]
        yc = ytc.reshape(128, 2, 8, 512).transpose(2, 0, 1, 3).reshape(1024, 1024)
        Y[b, half * TC:(half + 1) * TC, :] = yc.T
    return Y


# revision 51
# speedup vs baseline: 1.0332x; 1.0332x over previous
"""Trainium2 Bass kernel for nn_LDS_LR: low-rank LDS + AR low-rank correction.

Math (per batch b):
    Bu   = X @ B1 @ B2                      # [T, N] rank-64 input projection
    h_t  = A * h_{t-1} + Bu_t               # diagonal recurrence, h_{-1} = h0
    lds  = H @ C1 @ C2                      # [T, O] rank-64 output projection
    proj = einsum('ti,rik->trk', X, M1)     # [T, R, KX]
    ar_t = sum_k M2[:,:,k] @ proj[t-k,:,k]  # AR with KX=5 taps
    Y    = lds + ar

Sharding: 8 cores = 4 batches x 2 sequence halves (1024 steps each).

v2 design notes (vs the v1 carry-scan kernel):
  * The chunk-boundary carry h_1023 = sum_s A^(1023-s) Bu_prev[s] + A^1024 h0
    is computed WITHOUT prefix scans: the decay-power matrix
    APn[n,s] = A[n]^(1023-s) is precomputed host-side (weights-only
    transform), and one fused multiply+reduce (tensor_tensor_reduce on DVE /
    scalar_tensor_tensor+accum on Pool) per (n-tile, t-block) yields the
    carry.  This removes 8 full-length scans (~18.6us of DVE).
  * All matmul loops are k-outer so consecutive matmuls share a stationary
    operand (LDWEIGHTS serializes against same-row-group matmuls; halving
    loads keeps the PE column-issue cadence near peak).  PE emission is
    software-pipelined so the PE never waits on DVE scans (avoids HAM
    re-throttle windows triggered by PE idle gaps).
  * Own-half scans run per 512-col block, block-0 first, so CH1/Y/output-DMA
    of block 0 overlap block-1 scans.  A few block-1 scans run on the Pool
    engine in parallel with DVE.
  * All DRAM operands are host-packed into [128, *] row-major blobs so each
    loads with 1-2 large DMAs (11 input DMAs total, on the two hardware DGE
    queues: SP and Activation).  Output is bf16 (host casts back to fp32).
"""

import contextlib
import ctypes
import os
import sys
import types

import numpy as np
from contextlib import ExitStack

import concourse.bass as bass
import concourse.tile as tile
from concourse import bacc, mybir
from concourse.bass_utils import run_bass_kernel_spmd


def _install_ntff_hook():
    """Provide antenv.axon_hooks.get_axon_ntff_profile_hook if the image
    lacks it, driving NTFF capture via the libaxon_pjrt C ABI directly."""
    try:
        from antenv.axon_hooks import get_axon_ntff_profile_hook  # noqa: F401
        return
    except ImportError:
        pass
    so_path = "/opt/axon/libaxon_pjrt.so"
    hook = None
    if os.path.exists(so_path):
        lib = ctypes.CDLL(so_path)
        if hasattr(lib, "axon_start_nrt_profile"):
            lib.axon_start_nrt_profile.argtypes = [
                ctypes.POINTER(ctypes.c_int64), ctypes.c_size_t]
            lib.axon_start_nrt_profile.restype = ctypes.c_int64
            lib.axon_stop_nrt_profile.argtypes = [ctypes.c_char_p]
            lib.axon_stop_nrt_profile.restype = ctypes.c_int64

            @contextlib.contextmanager
            def _hook(output_dir, device_ids):
                import jax
                jax.devices()
                if device_ids:
                    ids = (ctypes.c_int64 * len(device_ids))(*device_ids)
                    rc = lib.axon_start_nrt_profile(ids, len(device_ids))
                else:
                    rc = lib.axon_start_nrt_profile(None, 0)
                if rc != 0:
                    raise RuntimeError(f"axon_start_nrt_profile rc={rc}")
                try:
                    yield
                finally:
                    n = lib.axon_stop_nrt_profile(str(output_dir).encode())
                    print(f"ntff profile: {n} file(s) -> {output_dir}",
                          file=sys.stderr)

            hook = _hook
    mod = types.ModuleType("antenv.axon_hooks")
    mod.get_axon_ntff_profile_hook = lambda: hook
    mod.set_axon_ntff_profile_hook = lambda h: None
    sys.modules["antenv.axon_hooks"] = mod


_install_ntff_hook()

DT = mybir.dt.float32
_MDT_NAME = os.environ.get("KERNEL_MDT", "bf16")
MDT = {"f32": mybir.dt.float32, "f32r": mybir.dt.float32r,
       "bf16": mybir.dt.bfloat16}[_MDT_NAME]
MNP = mybir.dt.np(MDT)
F32 = np.float32

OUT_BF16 = bool(int(os.environ.get("KERNEL_OUT_BF16", "1")))
# "1": stride-0 broadcast AP; "act": materialize A rows on the scalar engine;
# "0": materialize on the Pool engine (slow, debug only)
BCAST = os.environ.get("KERNEL_BCAST", "1")
# Pool engine cannot touch PSUM (BIR verifier), so offloading b1 scans to it
# requires a scalar-engine PSUM->SBUF staging copy per tile first.
GPS_SCAN = int(os.environ.get("KERNEL_GPS_SCAN", "0"))
USE_TTR = bool(int(os.environ.get("KERNEL_TTR", "1")))   # fused reduce for carry
SPLIT_SCAN = bool(int(os.environ.get("KERNEL_SPLIT_SCAN", "1")))
# ttr (tensor_tensor_reduce) FAULTS trn2 hardware (NRT_EXEC_UNIT_UNRECOVERABLE)
# despite passing CoreSim.  "pe" computes the carry with matmuls (G^T via
# identity-matmul, V = G^T @ AP2, then V*B2 summed by a ones-matmul), freeing
# the DVE for the own-half scans; "stt"/"scan" are DVE fallbacks.
CARRY = os.environ.get("KERNEL_CARRY", "pe")  # pe | ttr | stt | scan
INTERLEAVE = bool(int(os.environ.get("KERNEL_INTERLEAVE", "1")))
SCALQ = bool(int(os.environ.get("KERNEL_SCALQ", "1")))  # scalar-engine DMA queue
VECCOPY = bool(int(os.environ.get("KERNEL_VECCOPY", "1")))

ODT = MDT if OUT_BF16 else DT
ONP = mybir.dt.np(ODT)

B, T, D = 4, 2048, 1024
NST, R, KX, OUT = 1024, 64, 5, 1024
TC = 1024          # per-core chunk length
TBL = 512          # time block (one PSUM bank at fp32)

_CACHED_NC = None
LAST_RESULT = None  # BassKernelResults of the most recent run (for test.py)

MULT = mybir.AluOpType.mult
ADD = mybir.AluOpType.add


def _emit(ctx, tc, io):
    nc = tc.nc
    xo, xp, w1, b2, c1, w2, ap, avio, ident, yt = io

    wp = ctx.enter_context(tc.tile_pool(name="wp", bufs=1))
    xpool = ctx.enter_context(tc.tile_pool(name="xpool", bufs=1))
    hp = ctx.enter_context(tc.tile_pool(name="hp", bufs=1))
    pp = ctx.enter_context(tc.tile_pool(name="pp", bufs=1))
    yp = ctx.enter_context(tc.tile_pool(name="yp", bufs=1))
    # PSUM: pA (j/Y) 2 banks, pB (Bu) 4 banks, pC (G/CH1/tails) 2 banks = 8
    pA = ctx.enter_context(tc.tile_pool(name="pA", bufs=2, space="PSUM"))
    pB = ctx.enter_context(
        tc.tile_pool(name="pB", bufs=(4 if SPLIT_SCAN else 2), space="PSUM"))
    pC = ctx.enter_context(tc.tile_pool(name="pC", bufs=2, space="PSUM"))

    # ---------------- input DMAs (three queues, deadline-ordered) -----------
    # sync (Q1, earliest start): lnaio, xp quarters, b2  (+ y outs later)
    # gpsimd (Q0): w1g, xo quarters, ident
    # scalar (Q10, latest start): w1r, c1, avio, w2
    # AP decay powers are built ON DEVICE from lnA (Pool mult + Scalar exp),
    # saving 2MB of front-end DMA.
    lnaio = wp.tile([128, 1160], DT, tag="lnaio", name="lnaio")
    if CARRY == "pe":
        nc.sync.dma_start(lnaio[:], ap[:])
    xpts = [xpool.tile([128, 2048], MDT, tag=f"xpq{i}", name=f"xpq{i}")
            for i in range(4)]
    for i in range(4):
        nc.sync.dma_start(xpts[i][:], xp[:, i * 2048:(i + 1) * 2048])
    b2sb = wp.tile([64, 1024], MDT, tag="b2", name="b2sb")
    nc.sync.dma_start(b2sb[:], b2[:])

    w1gsb = wp.tile([128, 1024], MDT, tag="w1g", name="w1gsb")
    nc.gpsimd.dma_start(w1gsb[:], w1[:, 0:1024])
    xots = [xpool.tile([128, 2048], MDT, tag=f"xoq{i}", name=f"xoq{i}")
            for i in range(4)]
    for i in range(4):
        nc.gpsimd.dma_start(xots[i][:], xo[:, i * 2048:(i + 1) * 2048])
    identsb = wp.tile([64, 65], MDT, tag="ident", name="identsb")
    nc.gpsimd.dma_start(identsb[:], ident[:])

    w1rsb = wp.tile([128, 2048], MDT, tag="w1r", name="w1rsb")
    nc.scalar.dma_start(w1rsb[:], w1[:, 1024:3072])
    c1sb = wp.tile([128, 512], MDT, tag="c1", name="c1sb")
    nc.scalar.dma_start(c1sb[:], c1[:])
    aviosb = wp.tile([128, 16], DT, tag="avio", name="aviosb")
    nc.scalar.dma_start(aviosb[:], avio[:])
    w2sb = wp.tile([128, 3072], MDT, tag="w2", name="w2sb")
    nc.scalar.dma_start(w2sb[:], w2[:])

    if CARRY == "pe":
        # on-device AP2 build: ap_st[p, n] = exp(lnA[n] * (1023 - 128*st - p)).
        # lnaio row 0 cols 0:1024 = lnA (fp32); cols 1024:1032 = per-partition
        # exponents e_st[p] = 1023 - 128*st - p.  One ACT op per s-tile:
        # out = Exp(in * scale), in = partition-broadcast lnA row, scale=e_st.
        apts = [xpool.tile([128, 1024], MDT, tag=f"apq{st}", name=f"apq{st}")
                for st in range(8)]
        # replicate the lnA row to all partitions with a K=1 matmul
        # (ones[1,128]^T @ lnA[1,1024]), evict once, then 8 Exp builds.
        lnarep = wp.tile([128, 1024], DT, tag="lnarep", name="lnarep")
        for h in range(2):
            lr_ps = pA.tile([128, TBL], DT, tag="pa", name=f"lr_ps{h}")
            nc.tensor.matmul(lr_ps[:], lnaio[0:1, 1032:1160],
                             lnaio[0:1, h * TBL:(h + 1) * TBL],
                             start=True, stop=True)
            nc.scalar.copy(lnarep[:, h * TBL:(h + 1) * TBL], lr_ps[:])
        for st in range(8):
            nc.scalar.activation(
                apts[st][:], lnarep[:],
                mybir.ActivationFunctionType.Exp,
                scale=lnaio[:, 1024 + st:1025 + st])

        def apt(n, tb):  # AP slice for (s-tile, half) [128, 512]
            return apts[n][:, tb * 512:(tb + 1) * 512]
    else:
        apa = xpool.tile([128, 4096], MDT, tag="apa", name="apa")
        apb = xpool.tile([128, 4096], MDT, tag="apb", name="apb")
        nc.sync.dma_start(apa[:], ap[:, 0:4096])
        nc.sync.dma_start(apb[:], ap[:, 4096:8192])

        def apt(n, tb):  # AP slice for (n-tile, t-block) [128, 512]
            t = apa if n < 4 else apb
            return t[:, (n % 4) * 1024 + tb * 512:(n % 4) * 1024 + (tb + 1) * 512]

    def xpt(k):  # xp k-tile [128, 1024]
        return xpts[k // 2][:, (k % 2) * 1024:(k % 2 + 1) * 1024]

    def xot(k):
        return xots[k // 2][:, (k % 2) * 1024:(k % 2 + 1) * 1024]

    def w1t(k, lo, hi):  # W1 k-tile column slice (w1g: cols 0:128, w1r: rest)
        if hi <= 128:
            return w1gsb[:, k * 128 + lo:k * 128 + hi]
        assert lo >= 128
        return w1rsb[:, k * 256 + lo - 128:k * 256 + hi - 128]

    def w2t(m, o):  # W2 stationary for (m-tile, o-tile) [128, 128]
        return w2sb[:, m * 1024 + o * 128:m * 1024 + (o + 1) * 128]

    # A-broadcast for the scans: stride-0 view of avio column n (fallback:
    # materialized [128, TC] tiles on the Pool engine).
    if BCAST == "1":
        def abv(n, tb):
            return aviosb[:, n:n + 1].broadcast_to((128, 512))

        def abv_full(n):
            return aviosb[:, n:n + 1].broadcast_to((128, TC))
    else:
        ABW = 512 if (SPLIT_SCAN and CARRY != "scan") else TC
        ones = wp.tile([128, ABW], DT, tag="ones", name="ones")
        absb = []
        if BCAST == "act":
            nc.vector.memset(ones[:], 1.0)
            for n in range(8):
                ab = wp.tile([128, ABW], DT, tag=f"ab{n}", name=f"ab{n}")
                nc.scalar.mul(ab[:], ones[:], aviosb[:, n:n + 1])
                absb.append(ab)
        else:
            nc.gpsimd.memset(ones[:], 1.0)
            for n in range(8):
                ab = wp.tile([128, ABW], DT, tag=f"ab{n}", name=f"ab{n}")
                nc.gpsimd.tensor_scalar_mul(ab[:], ones[:], aviosb[:, n:n + 1])
                absb.append(ab)

        def abv(n, tb):
            return absb[n][:, 0:512]

        def abv_full(n):
            return absb[n][:]

    # ---------------- G_prev = (X_prev @ B1)^T and P_ext j0 ------------------
    gsb = wp.tile([64, 1024], MDT, tag="gprev", name="gsb")
    g_ps = [pC.tile([64, TBL], DT, tag="pc", name=f"g_ps{t}") for t in range(2)]
    PW = 4 + TC + 4
    pext = [pp.tile([128, PW], MDT, tag=f"pext{j}", name=f"pext{j}")
            for j in range(3)]
    j_ps = [pA.tile([128, TBL], DT, tag="pa", name=f"j0_ps{t}")
            for t in range(2)]

    def mm_g(k):
        for t in range(2):
            nc.tensor.matmul(g_ps[t][:], w1t(k, 0, 64),
                             xpt(k)[:, t * TBL:(t + 1) * TBL],
                             start=(k == 0), stop=(k == 7))

    def mm_j0(k):
        for t in range(2):
            nc.tensor.matmul(j_ps[t][:], w1t(k, 0, 128),
                             xot(k)[:, t * TBL:(t + 1) * TBL],
                             start=(k == 0), stop=(k == 7))

    if CARRY == "pe":
        # prev-half G first (xp rides the fast sync queue) so the carry chain
        # completes early; j0 follows, paced by the xo quarters on Q0.
        for k in range(8):
            mm_g(k)
        for k in range(8):
            mm_j0(k)
    elif INTERLEAVE:
        for k in range(8):
            mm_g(k)
        for k in range(8):
            mm_j0(k)
    else:
        for t in range(2):
            for k in range(8):
                nc.tensor.matmul(g_ps[t][:], w1t(k, 0, 64),
                                 xpt(k)[:, t * TBL:(t + 1) * TBL],
                                 start=(k == 0), stop=(k == 7))
        for t in range(2):
            for k in range(8):
                nc.tensor.matmul(j_ps[t][:], w1t(k, 0, 128),
                                 xot(k)[:, t * TBL:(t + 1) * TBL],
                                 start=(k == 0), stop=(k == 7))
    if CARRY == "pe":
        for t in range(2):
            nc.scalar.copy(gsb[:, t * TBL:(t + 1) * TBL], g_ps[t][:])
        for t in range(2):
            nc.scalar.copy(pext[0][:, 4 + t * TBL:4 + (t + 1) * TBL],
                           j_ps[t][:])
    else:
        for t in range(2):
            nc.scalar.copy(pext[0][:, 4 + t * TBL:4 + (t + 1) * TBL],
                           j_ps[t][:])
        for t in range(2):
            nc.scalar.copy(gsb[:, t * TBL:(t + 1) * TBL], g_ps[t][:])

    # ---------------- carry: Bu_prev + fused AP-weighted reduce --------------
    # iown[n] = sum_s A^(1023-s) Bu_prev[n,s] + ioff[n]; ioff host-folds
    # A^1024 h0 (half-1) / h0 (half-0, xp=0 so the sum vanishes).
    iown = [wp.tile([128, 1], DT, tag=f"iown{n}", name=f"iown{n}")
            for n in range(8)]
    tmpv = wp.tile([128, 512], MDT, tag="tmpv", name="tmpv")
    gacc = [wp.tile([128, 1], DT, tag=f"gacc{i}", name=f"gacc{i}")
            for i in range(1)]
    cstmp = (wp.tile([128, TC], DT, tag="cstmp", name="cstmp")
             if CARRY == "scan" else None)

    def emit_bup(n):
        if SPLIT_SCAN:
            bu = [pB.tile([128, TBL], DT, tag="pb", name=f"bup{n}_{t}")
                  for t in range(2)]
        else:
            bun = pB.tile([128, TC], DT, tag="pb", name=f"bup{n}")
            bu = [bun[:, 0:TBL], bun[:, TBL:TC]]
        for t in range(2):
            nc.tensor.matmul(bu[t][:], b2sb[:, n * 128:(n + 1) * 128],
                             gsb[:, t * TBL:(t + 1) * TBL],
                             start=True, stop=True)
        ioffap = aviosb[:, 8 + n:9 + n]
        if CARRY == "ttr":
            # DVE: fused (bu*AP) elementwise + chained reduce, initial=ioff
            nc.vector.tensor_tensor_reduce(
                tmpv[:], bu[0][:], apt(n, 0), 1.0, ioffap, MULT, ADD,
                accum_out=gacc[0][:])
            nc.vector.tensor_tensor_reduce(
                tmpv[:], bu[1][:], apt(n, 1), 1.0, gacc[0][:], MULT, ADD,
                accum_out=iown[n][:])
        elif CARRY == "stt":
            # stt mult + plain reduce + adds
            XAX = mybir.AxisListType.X
            nc.vector.scalar_tensor_tensor(
                tmpv[:], bu[0][:], 1.0, apt(n, 0), MULT, MULT)
            nc.vector.tensor_reduce(gacc[0][:], tmpv[:], XAX, ADD)
            nc.vector.scalar_tensor_tensor(
                tmpv[:], bu[1][:], 1.0, apt(n, 1), MULT, MULT)
            nc.vector.tensor_reduce(iown[n][:], tmpv[:], XAX, ADD)
            nc.vector.tensor_scalar_add(iown[n][:], iown[n][:], gacc[0][:])
            nc.vector.tensor_scalar_add(iown[n][:], iown[n][:], ioffap)
        else:
            # v1-style carry scans (fp32 temp), iown = cs[-1] + ioff
            nc.vector.tensor_tensor_scan(
                cstmp[:, 0:TBL], abv(n, 0), bu[0][:], 0.0, MULT, ADD)
            nc.vector.tensor_tensor_scan(
                cstmp[:, TBL:TC], abv(n, 1), bu[1][:],
                cstmp[:, TBL - 1:TBL], MULT, ADD)
            nc.vector.tensor_scalar_add(iown[n][:], cstmp[:, TC - 1:TC],
                                        ioffap)

    if CARRY == "pe":
        # -------- carry on the PE: G^T tiles, V = G^T @ AP2, reduce ---------
        # gt[:, st*64:(st+1)*64] = (G[:, st*128:(st+1)*128])^T via identity
        # matmuls, all into ONE psum bank -> single eviction.
        gtsb = wp.tile([128, 512], MDT, tag="gt", name="gtsb")
        gt_ps = pA.tile([128, 512], DT, tag="pa", name="gt_ps")
        for st in range(8):
            nc.tensor.matmul(gt_ps[:, st * 64:(st + 1) * 64],
                             gsb[:, st * 128:(st + 1) * 128],
                             identsb[0:64, 0:64], start=True, stop=True)
        nc.scalar.copy(gtsb[:], gt_ps[:])
        # V[r, n] = sum_s G[r, s] * A[n]^(1023-s)
        v_ps = [pC.tile([64, TBL], DT, tag="pc", name=f"v_ps{nh}")
                for nh in range(2)]
        for st in range(8):
            for nh in range(2):
                nc.tensor.matmul(v_ps[nh][:],
                                 gtsb[:, st * 64:(st + 1) * 64], apt(st, nh),
                                 start=(st == 0), stop=(st == 7))
        # E = V * B2 (DVE), then iown[n-tile] = ones-matmul over r + ioff
        esb = wp.tile([64, 1024], MDT, tag="esb", name="esb")
        for nh in range(2):
            nc.vector.scalar_tensor_tensor(
                esb[:, nh * TBL:(nh + 1) * TBL], v_ps[nh][:], 1.0,
                b2sb[:, nh * TBL:(nh + 1) * TBL], MULT, MULT)
        iown_ps = pB.tile([128, 8], DT, tag="pb", name="iown_ps")

        def emit_iown_mm():
            for n in range(8):
                nc.tensor.matmul(iown_ps[:, n:n + 1],
                                 esb[:, n * 128:(n + 1) * 128],
                                 identsb[0:64, 64:65], start=True, stop=True)
            for n in range(8):
                nc.vector.tensor_scalar_add(
                    iown[n][:], iown_ps[:, n:n + 1], aviosb[:, 8 + n:9 + n])
    else:
        for n in range(8):
            emit_bup(n)

    # ---------------- own Bu + scans (block-pipelined) -----------------------
    hsb = [hp.tile([128, TC], MDT, tag=f"h{n}", name=f"h{n}") for n in range(8)]

    buo_full = {}

    def emit_buo(n, tb):
        if not SPLIT_SCAN:
            # unsplit fallback: both halves into one [128, TC] tile, then a
            # single full-length scan when tb==1 is requested.
            if tb == 0:
                bun = pB.tile([128, TC], DT, tag="pb", name=f"buo{n}")
                buo_full[n] = bun
                for t in range(2):
                    nc.tensor.matmul(
                        bun[:, t * TBL:(t + 1) * TBL],
                        b2sb[:, n * 128:(n + 1) * 128],
                        pext[0][0:64, 4 + t * TBL:4 + (t + 1) * TBL],
                        start=True, stop=True)
            else:
                nc.vector.tensor_tensor_scan(
                    hsb[n][:], abv_full(n), buo_full[n][:], iown[n][:],
                    MULT, ADD)
            return
        bu = pB.tile([128, TBL], DT, tag="pb", name=f"buo{n}_{tb}")
        nc.tensor.matmul(bu[:], b2sb[:, n * 128:(n + 1) * 128],
                         pext[0][0:64, 4 + tb * TBL:4 + (tb + 1) * TBL],
                         start=True, stop=True)
        if tb == 0:
            nc.vector.tensor_tensor_scan(
                hsb[n][:, 0:TBL], abv(n, 0), bu[:], iown[n][:], MULT, ADD)
        elif n < GPS_SCAN:
            # Pool can't read PSUM: stage bu to SBUF via the scalar engine.
            stg = wp.tile([128, TBL], DT, tag=f"stg{n}", name=f"stg{n}")
            nc.scalar.copy(stg[:], bu[:])
            nc.gpsimd.tensor_tensor_scan(
                hsb[n][:, TBL:TC], abv(n, 1), stg[:],
                hsb[n][:, TBL - 1:TBL], MULT, ADD)
        else:
            nc.vector.tensor_tensor_scan(
                hsb[n][:, TBL:TC], abv(n, 1), bu[:],
                hsb[n][:, TBL - 1:TBL], MULT, ADD)

    # P_ext j1/j2 with the boundary-tail matmuls folded into the same
    # stationary (tap ka -> rows 0:64 shifted by ka, kb -> rows 64:128 by kb).
    def emit_j(j, klo, khi, jps, tl):
        if INTERLEAVE:
            for k in range(klo, khi):
                st = w1t(k, j * 128, (j + 1) * 128)
                for t in range(2):
                    nc.tensor.matmul(jps[t][:], st,
                                     xot(k)[:, t * TBL:(t + 1) * TBL],
                                     start=(k == 0), stop=(k == 7))
                nc.tensor.matmul(tl[:], st, xpt(k)[:, TC - 4:TC],
                                 start=(k == 0), stop=(k == 7))
        else:
            for t in range(2):
                for k in range(klo, khi):
                    nc.tensor.matmul(jps[t][:], w1t(k, j * 128, (j + 1) * 128),
                                     xot(k)[:, t * TBL:(t + 1) * TBL],
                                     start=(k == 0), stop=(k == 7))
            for k in range(klo, khi):
                nc.tensor.matmul(tl[:], w1t(k, j * 128, (j + 1) * 128),
                                 xpt(k)[:, TC - 4:TC],
                                 start=(k == 0), stop=(k == 7))

    def evict_j(j, jps, tl):
        ka, kb = 2 * j - 1, 2 * j
        for t in range(2):
            nc.scalar.copy(
                pext[j][0:64, 4 + ka + t * TBL:4 + ka + (t + 1) * TBL],
                jps[t][0:64, :])
            nc.scalar.copy(
                pext[j][64:128, 4 + kb + t * TBL:4 + kb + (t + 1) * TBL],
                jps[t][64:128, :])
        nc.scalar.copy(pext[j][0:64, 4:4 + ka], tl[0:64, 4 - ka:4])
        nc.scalar.copy(pext[j][64:128, 4:4 + kb], tl[64:128, 4 - kb:4])

    j1ps = [pA.tile([128, TBL], DT, tag="pa", name=f"j1_ps{t}")
            for t in range(2)]
    tl1 = pC.tile([128, 4], DT, tag="pc", name="tl1")
    if CARRY == "pe":
        emit_j(1, 0, 4, j1ps, tl1)
        emit_iown_mm()
        emit_buo(0, 0)
        emit_buo(1, 0)
        emit_j(1, 4, 8, j1ps, tl1)
        evict_j(1, j1ps, tl1)
        emit_buo(2, 0)
        emit_buo(3, 0)
        j2ps = [pA.tile([128, TBL], DT, tag="pa", name=f"j2_ps{t}")
                for t in range(2)]
        tl2 = pC.tile([128, 4], DT, tag="pc", name="tl2")
        emit_j(2, 0, 4, j2ps, tl2)
        emit_buo(4, 0)
        emit_buo(5, 0)
        emit_j(2, 4, 8, j2ps, tl2)
        evict_j(2, j2ps, tl2)
        emit_buo(6, 0)
        emit_buo(7, 0)
    elif INTERLEAVE:
        emit_buo(0, 0)
        emit_buo(1, 0)
        emit_j(1, 0, 4, j1ps, tl1)
        emit_buo(2, 0)
        emit_buo(3, 0)
        emit_j(1, 4, 8, j1ps, tl1)
        evict_j(1, j1ps, tl1)
        emit_buo(4, 0)
        emit_buo(5, 0)
        j2ps = [pA.tile([128, TBL], DT, tag="pa", name=f"j2_ps{t}")
                for t in range(2)]
        tl2 = pC.tile([128, 4], DT, tag="pc", name="tl2")
        emit_j(2, 0, 4, j2ps, tl2)
        emit_buo(6, 0)
        emit_buo(7, 0)
        emit_j(2, 4, 8, j2ps, tl2)
        evict_j(2, j2ps, tl2)
    else:
        emit_j(1, 0, 8, j1ps, tl1)
        evict_j(1, j1ps, tl1)
        j2ps = [pA.tile([128, TBL], DT, tag="pa", name=f"j2_ps{t}")
                for t in range(2)]
        tl2 = pC.tile([128, 4], DT, tag="pc", name="tl2")
        emit_j(2, 0, 8, j2ps, tl2)
        evict_j(2, j2ps, tl2)
        for n in range(8):
            emit_buo(n, 0)

    for n in range(4):
        emit_buo(n, 1)

    # ---------------- CH1 + Y per t-block, streamed out ----------------------
    ysb = [[yp.tile([128, 4 * TBL], ODT, tag=f"y{tb}{g}", name=f"y{tb}{g}")
            for g in range(2)] for tb in range(2)]

    def emit_ch1(tb):
        cps = pC.tile([64, TBL], DT, tag="pc", name=f"c_ps{tb}")
        for n in range(8):
            nc.tensor.matmul(cps[:], c1sb[:, n * 64:(n + 1) * 64],
                             hsb[n][:, tb * TBL:(tb + 1) * TBL],
                             start=(n == 0), stop=(n == 7))
        nc.scalar.copy(pext[0][0:64, 4 + tb * TBL:4 + (tb + 1) * TBL], cps[:])

    def emit_y(tb):
        for o in range(8):
            yps = pA.tile([128, TBL], DT, tag="pa", name=f"y_ps{o}_{tb}")
            for m in range(3):
                nc.tensor.matmul(yps[:], w2t(m, o),
                                 pext[m][:, 4 + tb * TBL:4 + (tb + 1) * TBL],
                                 start=(m == 0), stop=(m == 2))
            g, oo = divmod(o, 4)
            dst = ysb[tb][g][:, oo * TBL:(oo + 1) * TBL]
            if (tb == 0 and o < 6) or not VECCOPY:
                nc.scalar.copy(dst, yps[:])
            else:
                nc.vector.tensor_copy(dst, yps[:])
            if oo == 3:
                nc.sync.dma_start(
                    yt[:, tb * 4096 + g * 2048:tb * 4096 + (g + 1) * 2048],
                    ysb[tb][g][:])

    if SPLIT_SCAN:
        emit_ch1(0)
        for n in range(4, 8):
            emit_buo(n, 1)
    else:
        for n in range(4, 8):
            emit_buo(n, 1)
        emit_ch1(0)
    emit_y(0)
    emit_ch1(1)
    emit_y(1)


def _build():
    nc = bacc.Bacc("TRN2", target_bir_lowering=False, debug=False,
                   num_devices=8)
    xo = nc.dram_tensor("xo", [128, 8192], MDT, kind="ExternalInput").ap()
    xp = nc.dram_tensor("xp", [128, 8192], MDT, kind="ExternalInput").ap()
    w1 = nc.dram_tensor("w1", [128, 3072], MDT, kind="ExternalInput").ap()
    b2 = nc.dram_tensor("b2", [64, 1024], MDT, kind="ExternalInput").ap()
    c1 = nc.dram_tensor("c1", [128, 512], MDT, kind="ExternalInput").ap()
    w2 = nc.dram_tensor("w2", [128, 3072], MDT, kind="ExternalInput").ap()
    if CARRY == "pe":
        ap = nc.dram_tensor("ap", [128, 1160], DT, kind="ExternalInput").ap()
    else:
        ap = nc.dram_tensor("ap", [128, 8192], MDT, kind="ExternalInput").ap()
    avio = nc.dram_tensor("avio", [128, 16], DT, kind="ExternalInput").ap()
    ident = nc.dram_tensor("ident", [64, 65], MDT, kind="ExternalInput").ap()
    yt = nc.dram_tensor("yt", [128, 8192], ODT, kind="ExternalOutput").ap()

    with tile.TileContext(nc) as tc, ExitStack() as ctx:
        _emit(ctx, tc, (xo, xp, w1, b2, c1, w2, ap, avio, ident, yt))
    nc.compile()
    return nc


def _get_nc():
    global _CACHED_NC
    if _CACHED_NC is None:
        _CACHED_NC = _build()
    return _CACHED_NC


def _pack_kt(arr):
    """[1024, C] -> [128, 8*C] with blocks of 128 rows side by side."""
    C = arr.shape[1]
    return np.ascontiguousarray(
        arr.reshape(8, 128, C).transpose(1, 0, 2).reshape(128, 8 * C))


def kernel(inputs, h0, A, B1, B2, C1, C2, M1, M2):
    global LAST_RESULT
    X = np.asarray(inputs, dtype=F32)
    h0 = np.asarray(h0, dtype=F32)
    A64 = np.asarray(A, dtype=np.float64)

    W1 = np.concatenate(
        [np.asarray(B1, dtype=F32)]
        + [np.ascontiguousarray(np.asarray(M1, dtype=F32)[:, :, k].T)
           for k in range(KX)], axis=1)
    W2 = np.concatenate(
        [np.asarray(C2, dtype=F32)]
        + [np.ascontiguousarray(np.asarray(M2, dtype=F32)[:, :, k].T)
           for k in range(KX)], axis=0)
    w1kt = W1.astype(MNP).reshape(8, 128, 384)           # [k, p, c]
    w1p = np.concatenate([                               # [128, 1024 | 2048]
        np.ascontiguousarray(w1kt[:, :, 0:128].transpose(1, 0, 2)
                             .reshape(128, 1024)),
        np.ascontiguousarray(w1kt[:, :, 128:384].transpose(1, 0, 2)
                             .reshape(128, 2048))], axis=1)
    w2p = np.ascontiguousarray(
        W2.reshape(3, 128, 1024).transpose(1, 0, 2)
        .reshape(128, 3072).astype(MNP))                 # [128, 3072]
    b2c = np.ascontiguousarray(np.asarray(B2, dtype=F32).astype(MNP))
    c1p = _pack_kt(np.asarray(C1, dtype=F32).astype(MNP))  # [128, 512]

    # decay powers AP[n, s] = A[n]^(1023-s); for "pe" the device builds them
    # from lnA + per-partition exponents (saves 2MB of DMA)
    if CARRY == "pe":
        app = np.zeros((128, 1160), F32)
        app[0, 0:1024] = np.log(A64).astype(F32)
        for st in range(8):
            app[:, 1024 + st] = (TC - 1) - 128 * st - np.arange(128)
        app[0, 1032:1160] = 1.0
        app = np.ascontiguousarray(app)
    else:
        powers = (TC - 1) - np.arange(TC, dtype=np.float64)
        APm = np.exp(np.log(A64)[:, None] * powers[None, :])  # [1024,1024]
        app = _pack_kt(APm.astype(F32).astype(MNP))           # [128, 8192]
    identm = np.zeros((64, 65), F32)
    identm[:, 0:64] = np.eye(64, dtype=F32)
    identm[:, 64] = 1.0
    identm = np.ascontiguousarray(identm.astype(MNP))

    ioff_h0 = h0                                           # half 0: plain h0
    ioff_h1 = (A64 ** TC * h0.astype(np.float64)).astype(F32)  # A^1024 h0

    Xbf = X.astype(MNP)
    zeros_xp = np.zeros((128, 8192), MNP)

    def pack_x(b, sl):
        return _pack_kt(np.ascontiguousarray(Xbf[b, sl, :].T))

    in_maps = []
    for c in range(8):
        b, half = divmod(c, 2)
        xoc = pack_x(b, slice(half * TC, (half + 1) * TC))
        if half == 0:
            xpc, ioff = zeros_xp, ioff_h0
        else:
            xpc, ioff = pack_x(b, slice(0, TC)), ioff_h1
        avio = np.zeros((128, 16), F32)
        avio[:, 0:8] = np.asarray(A, dtype=F32).reshape(8, 128).T
        avio[:, 8:16] = ioff.reshape(8, 128).T
        in_maps.append({"xo": xoc, "xp": xpc, "w1": w1p, "b2": b2c,
                        "c1": c1p, "w2": w2p, "ap": app, "avio": avio,
                        "ident": identm})

    nc = _get_nc()
    trace = bool(int(os.environ.get("KERNEL_TRACE", "0")))
    LAST_RESULT = run_bass_kernel_spmd(nc, in_maps, core_ids=list(range(8)),
                                       trace=trace)
    Y = np.empty((B, T, OUT), F32)
    for c in range(8):
        b, half = divmod(c, 2)
        ytc = np.asarray(LAST_RESULT.results[c]["yt"], dtype=F32)
        # yt[p, tb*4096 + o*512 + t] -> Y_core[o*128+p, tb*512+t]
        yc = ytc.reshape(128, 2, 8, 512).transpose(2, 0, 1, 3).reshape(1024, 1024)
        Y[b, half * TC:(half + 1) * TC, :] = yc.T
    return Y


# revision 53
# speedup vs baseline: 1.0913x; 1.0562x over previous
"""Trainium2 Bass kernel for nn_LDS_LR: low-rank LDS + AR low-rank correction.

Math (per batch b):
    Bu   = X @ B1 @ B2                      # [T, N] rank-64 input projection
    h_t  = A * h_{t-1} + Bu_t               # diagonal recurrence, h_{-1} = h0
    lds  = H @ C1 @ C2                      # [T, O] rank-64 output projection
    proj = einsum('ti,rik->trk', X, M1)     # [T, R, KX]
    ar_t = sum_k M2[:,:,k] @ proj[t-k,:,k]  # AR with KX=5 taps
    Y    = lds + ar

Sharding: 8 cores = 4 batches x 2 sequence halves (1024 steps each).

v2 design notes (vs the v1 carry-scan kernel):
  * The chunk-boundary carry h_1023 = sum_s A^(1023-s) Bu_prev[s] + A^1024 h0
    is computed WITHOUT prefix scans: the decay-power matrix
    APn[n,s] = A[n]^(1023-s) is precomputed host-side (weights-only
    transform), and one fused multiply+reduce (tensor_tensor_reduce on DVE /
    scalar_tensor_tensor+accum on Pool) per (n-tile, t-block) yields the
    carry.  This removes 8 full-length scans (~18.6us of DVE).
  * All matmul loops are k-outer so consecutive matmuls share a stationary
    operand (LDWEIGHTS serializes against same-row-group matmuls; halving
    loads keeps the PE column-issue cadence near peak).  PE emission is
    software-pipelined so the PE never waits on DVE scans (avoids HAM
    re-throttle windows triggered by PE idle gaps).
  * Own-half scans run per 512-col block, block-0 first, so CH1/Y/output-DMA
    of block 0 overlap block-1 scans.  A few block-1 scans run on the Pool
    engine in parallel with DVE.
  * All DRAM operands are host-packed into [128, *] row-major blobs so each
    loads with 1-2 large DMAs (11 input DMAs total, on the two hardware DGE
    queues: SP and Activation).  Output is bf16 (host casts back to fp32).
"""

import contextlib
import ctypes
import os
import sys
import types

import numpy as np
from contextlib import ExitStack

import concourse.bass as bass
import concourse.tile as tile
from concourse import bacc, mybir
from concourse.bass_utils import run_bass_kernel_spmd


def _install_ntff_hook():
    """Provide antenv.axon_hooks.get_axon_ntff_profile_hook if the image
    lacks it, driving NTFF capture via the libaxon_pjrt C ABI directly."""
    try:
        from antenv.axon_hooks import get_axon_ntff_profile_hook  # noqa: F401
        return
    except ImportError:
        pass
    so_path = "/opt/axon/libaxon_pjrt.so"
    hook = None
    if os.path.exists(so_path):
        lib = ctypes.CDLL(so_path)
        if hasattr(lib, "axon_start_nrt_profile"):
            lib.axon_start_nrt_profile.argtypes = [
                ctypes.POINTER(ctypes.c_int64), ctypes.c_size_t]
            lib.axon_start_nrt_profile.restype = ctypes.c_int64
            lib.axon_stop_nrt_profile.argtypes = [ctypes.c_char_p]
            lib.axon_stop_nrt_profile.restype = ctypes.c_int64

            @contextlib.contextmanager
            def _hook(output_dir, device_ids):
                import jax
                jax.devices()
                if device_ids:
                    ids = (ctypes.c_int64 * len(device_ids))(*device_ids)
                    rc = lib.axon_start_nrt_profile(ids, len(device_ids))
                else:
                    rc = lib.axon_start_nrt_profile(None, 0)
                if rc != 0:
                    raise RuntimeError(f"axon_start_nrt_profile rc={rc}")
                try:
                    yield
                finally:
                    n = lib.axon_stop_nrt_profile(str(output_dir).encode())
                    print(f"ntff profile: {n} file(s) -> {output_dir}",
                          file=sys.stderr)

            hook = _hook
    mod = types.ModuleType("antenv.axon_hooks")
    mod.get_axon_ntff_profile_hook = lambda: hook
    mod.set_axon_ntff_profile_hook = lambda h: None
    sys.modules["antenv.axon_hooks"] = mod


_install_ntff_hook()

DT = mybir.dt.float32
_MDT_NAME = os.environ.get("KERNEL_MDT", "bf16")
MDT = {"f32": mybir.dt.float32, "f32r": mybir.dt.float32r,
       "bf16": mybir.dt.bfloat16}[_MDT_NAME]
MNP = mybir.dt.np(MDT)
F32 = np.float32

OUT_BF16 = bool(int(os.environ.get("KERNEL_OUT_BF16", "1")))
# "1": stride-0 broadcast AP; "act": materialize A rows on the scalar engine;
# "0": materialize on the Pool engine (slow, debug only)
BCAST = os.environ.get("KERNEL_BCAST", "1")
# Pool engine cannot touch PSUM (BIR verifier), so offloading b1 scans to it
# requires a scalar-engine PSUM->SBUF staging copy per tile first.
GPS_SCAN = int(os.environ.get("KERNEL_GPS_SCAN", "0"))
USE_TTR = bool(int(os.environ.get("KERNEL_TTR", "1")))   # fused reduce for carry
SPLIT_SCAN = bool(int(os.environ.get("KERNEL_SPLIT_SCAN", "1")))
# ttr (tensor_tensor_reduce) FAULTS trn2 hardware (NRT_EXEC_UNIT_UNRECOVERABLE)
# despite passing CoreSim.  "pe" computes the carry with matmuls (G^T via
# identity-matmul, V = G^T @ AP2, then V*B2 summed by a ones-matmul), freeing
# the DVE for the own-half scans; "stt"/"scan" are DVE fallbacks.
CARRY = os.environ.get("KERNEL_CARRY", "pe")  # pe | ttr | stt | scan
INTERLEAVE = bool(int(os.environ.get("KERNEL_INTERLEAVE", "1")))
SCALQ = bool(int(os.environ.get("KERNEL_SCALQ", "1")))  # scalar-engine DMA queue
VECCOPY = bool(int(os.environ.get("KERNEL_VECCOPY", "1")))

ODT = MDT if OUT_BF16 else DT
ONP = mybir.dt.np(ODT)

B, T, D = 4, 2048, 1024
NST, R, KX, OUT = 1024, 64, 5, 1024
TC = 1024          # per-core chunk length
TBL = 512          # time block (one PSUM bank at fp32)

_CACHED_NC = None
LAST_RESULT = None  # BassKernelResults of the most recent run (for test.py)

MULT = mybir.AluOpType.mult
ADD = mybir.AluOpType.add


def _emit(ctx, tc, io):
    nc = tc.nc
    xo, xp, w1, b2, c1, w2, ap, avio, ident, yt = io

    wp = ctx.enter_context(tc.tile_pool(name="wp", bufs=1))
    xpool = ctx.enter_context(tc.tile_pool(name="xpool", bufs=1))
    hp = ctx.enter_context(tc.tile_pool(name="hp", bufs=1))
    pp = ctx.enter_context(tc.tile_pool(name="pp", bufs=1))
    yp = ctx.enter_context(tc.tile_pool(name="yp", bufs=1))
    # PSUM: pA (j/Y) 2 banks, pB (Bu) 4 banks, pC (G/CH1/tails) 2 banks = 8
    pA = ctx.enter_context(tc.tile_pool(name="pA", bufs=2, space="PSUM"))
    pB = ctx.enter_context(
        tc.tile_pool(name="pB", bufs=(4 if SPLIT_SCAN else 2), space="PSUM"))
    pC = ctx.enter_context(tc.tile_pool(name="pC", bufs=2, space="PSUM"))

    # ---------------- input DMAs (three queues, deadline-ordered) -----------
    # sync (Q1, earliest start): lnaio, xp quarters, b2  (+ y outs later)
    # gpsimd (Q0): w1g, xo quarters, ident
    # scalar (Q10, latest start): w1r, c1, avio, w2
    # AP decay powers are built ON DEVICE from lnA (Pool mult + Scalar exp),
    # saving 2MB of front-end DMA.
    lnaio = wp.tile([128, 1160], DT, tag="lnaio", name="lnaio")
    if CARRY == "pe":
        nc.sync.dma_start(lnaio[:], ap[:])
    xpts = [xpool.tile([128, 2048], MDT, tag=f"xpq{i}", name=f"xpq{i}")
            for i in range(4)]
    for i in range(4):
        nc.sync.dma_start(xpts[i][:], xp[:, i * 2048:(i + 1) * 2048])
    b2sb = wp.tile([64, 1024], MDT, tag="b2", name="b2sb")
    nc.sync.dma_start(b2sb[:], b2[:])

    w1gsb = wp.tile([128, 1024], MDT, tag="w1g", name="w1gsb")
    nc.gpsimd.dma_start(w1gsb[:], w1[:, 0:1024])
    xots = [xpool.tile([128, 2048], MDT, tag=f"xoq{i}", name=f"xoq{i}")
            for i in range(4)]
    for i in range(4):
        nc.gpsimd.dma_start(xots[i][:], xo[:, i * 2048:(i + 1) * 2048])
    identsb = wp.tile([64, 65], MDT, tag="ident", name="identsb")
    nc.gpsimd.dma_start(identsb[:], ident[:])

    w1rsb = wp.tile([128, 2048], MDT, tag="w1r", name="w1rsb")
    nc.scalar.dma_start(w1rsb[:], w1[:, 1024:3072])
    c1sb = wp.tile([128, 512], MDT, tag="c1", name="c1sb")
    nc.scalar.dma_start(c1sb[:], c1[:])
    aviosb = wp.tile([128, 16], DT, tag="avio", name="aviosb")
    nc.scalar.dma_start(aviosb[:], avio[:])
    w2sb = wp.tile([128, 3072], MDT, tag="w2", name="w2sb")
    nc.scalar.dma_start(w2sb[:], w2[:])

    if CARRY == "pe":
        # on-device AP2 build: ap_st[p, n] = exp(lnA[n] * (1023 - 128*st - p)).
        # lnaio row 0 cols 0:1024 = lnA (fp32); cols 1024:1032 = per-partition
        # exponents e_st[p] = 1023 - 128*st - p.  One ACT op per s-tile:
        # out = Exp(in * scale), in = partition-broadcast lnA row, scale=e_st.
        apts = [xpool.tile([128, 1024], MDT, tag=f"apq{st}", name=f"apq{st}")
                for st in range(8)]
        # replicate the lnA row to all partitions with a K=1 matmul
        # (ones[1,128]^T @ lnA[1,1024]), evict once, then 8 Exp builds.
        lnarep = wp.tile([128, 1024], DT, tag="lnarep", name="lnarep")
        for h in range(2):
            lr_ps = pA.tile([128, TBL], DT, tag="pa", name=f"lr_ps{h}")
            nc.tensor.matmul(lr_ps[:], lnaio[0:1, 1032:1160],
                             lnaio[0:1, h * TBL:(h + 1) * TBL],
                             start=True, stop=True)
            nc.scalar.copy(lnarep[:, h * TBL:(h + 1) * TBL], lr_ps[:])
        for st in range(8):
            nc.scalar.activation(
                apts[st][:], lnarep[:],
                mybir.ActivationFunctionType.Exp,
                scale=lnaio[:, 1024 + st:1025 + st])

        def apt(n, tb):  # AP slice for (s-tile, half) [128, 512]
            return apts[n][:, tb * 512:(tb + 1) * 512]
    else:
        apa = xpool.tile([128, 4096], MDT, tag="apa", name="apa")
        apb = xpool.tile([128, 4096], MDT, tag="apb", name="apb")
        nc.sync.dma_start(apa[:], ap[:, 0:4096])
        nc.sync.dma_start(apb[:], ap[:, 4096:8192])

        def apt(n, tb):  # AP slice for (n-tile, t-block) [128, 512]
            t = apa if n < 4 else apb
            return t[:, (n % 4) * 1024 + tb * 512:(n % 4) * 1024 + (tb + 1) * 512]

    def xpt(k):  # xp k-tile [128, 1024]
        return xpts[k // 2][:, (k % 2) * 1024:(k % 2 + 1) * 1024]

    def xot(k):
        return xots[k // 2][:, (k % 2) * 1024:(k % 2 + 1) * 1024]

    def w1t(k, lo, hi):  # W1 k-tile column slice (w1g: cols 0:128, w1r: rest)
        if hi <= 128:
            return w1gsb[:, k * 128 + lo:k * 128 + hi]
        assert lo >= 128
        return w1rsb[:, k * 256 + lo - 128:k * 256 + hi - 128]

    def w2t(m, o):  # W2 stationary for (m-tile, o-tile) [128, 128]
        return w2sb[:, m * 1024 + o * 128:m * 1024 + (o + 1) * 128]

    # A-broadcast for the scans: stride-0 view of avio column n (fallback:
    # materialized [128, TC] tiles on the Pool engine).
    if BCAST == "1":
        def abv(n, tb):
            return aviosb[:, n:n + 1].broadcast_to((128, 512))

        def abv_full(n):
            return aviosb[:, n:n + 1].broadcast_to((128, TC))
    else:
        ABW = 512 if (SPLIT_SCAN and CARRY != "scan") else TC
        ones = wp.tile([128, ABW], DT, tag="ones", name="ones")
        absb = []
        if BCAST == "act":
            nc.vector.memset(ones[:], 1.0)
            for n in range(8):
                ab = wp.tile([128, ABW], DT, tag=f"ab{n}", name=f"ab{n}")
                nc.scalar.mul(ab[:], ones[:], aviosb[:, n:n + 1])
                absb.append(ab)
        else:
            nc.gpsimd.memset(ones[:], 1.0)
            for n in range(8):
                ab = wp.tile([128, ABW], DT, tag=f"ab{n}", name=f"ab{n}")
                nc.gpsimd.tensor_scalar_mul(ab[:], ones[:], aviosb[:, n:n + 1])
                absb.append(ab)

        def abv(n, tb):
            return absb[n][:, 0:512]

        def abv_full(n):
            return absb[n][:]

    # ---------------- G_prev = (X_prev @ B1)^T and P_ext j0 ------------------
    gsb = wp.tile([64, 1024], MDT, tag="gprev", name="gsb")
    g_ps = [pC.tile([64, TBL], DT, tag="pc", name=f"g_ps{t}") for t in range(2)]
    PW = 4 + TC + 4
    pext = [pp.tile([128, PW], MDT, tag=f"pext{j}", name=f"pext{j}")
            for j in range(3)]
    j_ps = [pA.tile([128, TBL], DT, tag="pa", name=f"j0_ps{t}")
            for t in range(2)]

    def mm_g(k):
        for t in range(2):
            nc.tensor.matmul(g_ps[t][:], w1t(k, 0, 64),
                             xpt(k)[:, t * TBL:(t + 1) * TBL],
                             start=(k == 0), stop=(k == 7))

    def mm_j0(k):
        for t in range(2):
            nc.tensor.matmul(j_ps[t][:], w1t(k, 0, 128),
                             xot(k)[:, t * TBL:(t + 1) * TBL],
                             start=(k == 0), stop=(k == 7))

    if CARRY == "pe":
        # interleave G (xp via Q1) and j0 (xo via Q0) per k so the PE drains
        # both DMA queues in parallel during the load-paced front end
        for k in range(8):
            mm_g(k)
            mm_j0(k)
    elif INTERLEAVE:
        for k in range(8):
            mm_g(k)
        for k in range(8):
            mm_j0(k)
    else:
        for t in range(2):
            for k in range(8):
                nc.tensor.matmul(g_ps[t][:], w1t(k, 0, 64),
                                 xpt(k)[:, t * TBL:(t + 1) * TBL],
                                 start=(k == 0), stop=(k == 7))
        for t in range(2):
            for k in range(8):
                nc.tensor.matmul(j_ps[t][:], w1t(k, 0, 128),
                                 xot(k)[:, t * TBL:(t + 1) * TBL],
                                 start=(k == 0), stop=(k == 7))
    if CARRY == "pe":
        for t in range(2):
            nc.scalar.copy(gsb[:, t * TBL:(t + 1) * TBL], g_ps[t][:])
        for t in range(2):
            nc.scalar.copy(pext[0][:, 4 + t * TBL:4 + (t + 1) * TBL],
                           j_ps[t][:])
    else:
        for t in range(2):
            nc.scalar.copy(pext[0][:, 4 + t * TBL:4 + (t + 1) * TBL],
                           j_ps[t][:])
        for t in range(2):
            nc.scalar.copy(gsb[:, t * TBL:(t + 1) * TBL], g_ps[t][:])

    # ---------------- carry: Bu_prev + fused AP-weighted reduce --------------
    # iown[n] = sum_s A^(1023-s) Bu_prev[n,s] + ioff[n]; ioff host-folds
    # A^1024 h0 (half-1) / h0 (half-0, xp=0 so the sum vanishes).
    iown = [wp.tile([128, 1], DT, tag=f"iown{n}", name=f"iown{n}")
            for n in range(8)]
    tmpv = wp.tile([128, 512], MDT, tag="tmpv", name="tmpv")
    gacc = [wp.tile([128, 1], DT, tag=f"gacc{i}", name=f"gacc{i}")
            for i in range(1)]
    cstmp = (wp.tile([128, TC], DT, tag="cstmp", name="cstmp")
             if CARRY == "scan" else None)

    def emit_bup(n):
        if SPLIT_SCAN:
            bu = [pB.tile([128, TBL], DT, tag="pb", name=f"bup{n}_{t}")
                  for t in range(2)]
        else:
            bun = pB.tile([128, TC], DT, tag="pb", name=f"bup{n}")
            bu = [bun[:, 0:TBL], bun[:, TBL:TC]]
        for t in range(2):
            nc.tensor.matmul(bu[t][:], b2sb[:, n * 128:(n + 1) * 128],
                             gsb[:, t * TBL:(t + 1) * TBL],
                             start=True, stop=True)
        ioffap = aviosb[:, 8 + n:9 + n]
        if CARRY == "ttr":
            # DVE: fused (bu*AP) elementwise + chained reduce, initial=ioff
            nc.vector.tensor_tensor_reduce(
                tmpv[:], bu[0][:], apt(n, 0), 1.0, ioffap, MULT, ADD,
                accum_out=gacc[0][:])
            nc.vector.tensor_tensor_reduce(
                tmpv[:], bu[1][:], apt(n, 1), 1.0, gacc[0][:], MULT, ADD,
                accum_out=iown[n][:])
        elif CARRY == "stt":
            # stt mult + plain reduce + adds
            XAX = mybir.AxisListType.X
            nc.vector.scalar_tensor_tensor(
                tmpv[:], bu[0][:], 1.0, apt(n, 0), MULT, MULT)
            nc.vector.tensor_reduce(gacc[0][:], tmpv[:], XAX, ADD)
            nc.vector.scalar_tensor_tensor(
                tmpv[:], bu[1][:], 1.0, apt(n, 1), MULT, MULT)
            nc.vector.tensor_reduce(iown[n][:], tmpv[:], XAX, ADD)
            nc.vector.tensor_scalar_add(iown[n][:], iown[n][:], gacc[0][:])
            nc.vector.tensor_scalar_add(iown[n][:], iown[n][:], ioffap)
        else:
            # v1-style carry scans (fp32 temp), iown = cs[-1] + ioff
            nc.vector.tensor_tensor_scan(
                cstmp[:, 0:TBL], abv(n, 0), bu[0][:], 0.0, MULT, ADD)
            nc.vector.tensor_tensor_scan(
                cstmp[:, TBL:TC], abv(n, 1), bu[1][:],
                cstmp[:, TBL - 1:TBL], MULT, ADD)
            nc.vector.tensor_scalar_add(iown[n][:], cstmp[:, TC - 1:TC],
                                        ioffap)

    if CARRY == "pe":
        # -------- carry on the PE: G^T tiles, V = G^T @ AP2, reduce ---------
        # gt[:, st*64:(st+1)*64] = (G[:, st*128:(st+1)*128])^T via identity
        # matmuls, all into ONE psum bank -> single eviction.
        gtsb = wp.tile([128, 512], MDT, tag="gt", name="gtsb")
        gt_ps = pA.tile([128, 512], DT, tag="pa", name="gt_ps")
        for st in range(8):
            nc.tensor.matmul(gt_ps[:, st * 64:(st + 1) * 64],
                             gsb[:, st * 128:(st + 1) * 128],
                             identsb[0:64, 0:64], start=True, stop=True)
        nc.scalar.copy(gtsb[:], gt_ps[:])
        # V[r, n] = sum_s G[r, s] * A[n]^(1023-s)
        v_ps = [pC.tile([64, TBL], DT, tag="pc", name=f"v_ps{nh}")
                for nh in range(2)]
        for st in range(8):
            for nh in range(2):
                nc.tensor.matmul(v_ps[nh][:],
                                 gtsb[:, st * 64:(st + 1) * 64], apt(st, nh),
                                 start=(st == 0), stop=(st == 7))
        # E = V * B2 (DVE), then iown[n-tile] = ones-matmul over r + ioff
        esb = wp.tile([64, 1024], MDT, tag="esb", name="esb")
        for nh in range(2):
            nc.vector.scalar_tensor_tensor(
                esb[:, nh * TBL:(nh + 1) * TBL], v_ps[nh][:], 1.0,
                b2sb[:, nh * TBL:(nh + 1) * TBL], MULT, MULT)
        iown_ps = pB.tile([128, 8], DT, tag="pb", name="iown_ps")

        def emit_iown_mm():
            for n in range(8):
                nc.tensor.matmul(iown_ps[:, n:n + 1],
                                 esb[:, n * 128:(n + 1) * 128],
                                 identsb[0:64, 64:65], start=True, stop=True)
            for n in range(8):
                nc.vector.tensor_scalar_add(
                    iown[n][:], iown_ps[:, n:n + 1], aviosb[:, 8 + n:9 + n])
    else:
        for n in range(8):
            emit_bup(n)

    # ---------------- own Bu + scans (block-pipelined) -----------------------
    hsb = [hp.tile([128, TC], MDT, tag=f"h{n}", name=f"h{n}") for n in range(8)]

    buo_full = {}

    def emit_buo(n, tb):
        if not SPLIT_SCAN:
            # unsplit fallback: both halves into one [128, TC] tile, then a
            # single full-length scan when tb==1 is requested.
            if tb == 0:
                bun = pB.tile([128, TC], DT, tag="pb", name=f"buo{n}")
                buo_full[n] = bun
                for t in range(2):
                    nc.tensor.matmul(
                        bun[:, t * TBL:(t + 1) * TBL],
                        b2sb[:, n * 128:(n + 1) * 128],
                        pext[0][0:64, 4 + t * TBL:4 + (t + 1) * TBL],
                        start=True, stop=True)
            else:
                nc.vector.tensor_tensor_scan(
                    hsb[n][:], abv_full(n), buo_full[n][:], iown[n][:],
                    MULT, ADD)
            return
        bu = pB.tile([128, TBL], DT, tag="pb", name=f"buo{n}_{tb}")
        nc.tensor.matmul(bu[:], b2sb[:, n * 128:(n + 1) * 128],
                         pext[0][0:64, 4 + tb * TBL:4 + (tb + 1) * TBL],
                         start=True, stop=True)
        if tb == 0:
            nc.vector.tensor_tensor_scan(
                hsb[n][:, 0:TBL], abv(n, 0), bu[:], iown[n][:], MULT, ADD)
        elif n < GPS_SCAN:
            # Pool can't read PSUM: stage bu to SBUF via the scalar engine.
            stg = wp.tile([128, TBL], DT, tag=f"stg{n}", name=f"stg{n}")
            nc.scalar.copy(stg[:], bu[:])
            nc.gpsimd.tensor_tensor_scan(
                hsb[n][:, TBL:TC], abv(n, 1), stg[:],
                hsb[n][:, TBL - 1:TBL], MULT, ADD)
        else:
            nc.vector.tensor_tensor_scan(
                hsb[n][:, TBL:TC], abv(n, 1), bu[:],
                hsb[n][:, TBL - 1:TBL], MULT, ADD)

    # P_ext j1/j2 with the boundary-tail matmuls folded into the same
    # stationary (tap ka -> rows 0:64 shifted by ka, kb -> rows 64:128 by kb).
    def emit_j(j, klo, khi, jps, tl):
        if INTERLEAVE:
            for k in range(klo, khi):
                st = w1t(k, j * 128, (j + 1) * 128)
                for t in range(2):
                    nc.tensor.matmul(jps[t][:], st,
                                     xot(k)[:, t * TBL:(t + 1) * TBL],
                                     start=(k == 0), stop=(k == 7))
                nc.tensor.matmul(tl[:], st, xpt(k)[:, TC - 4:TC],
                                 start=(k == 0), stop=(k == 7))
        else:
            for t in range(2):
                for k in range(klo, khi):
                    nc.tensor.matmul(jps[t][:], w1t(k, j * 128, (j + 1) * 128),
                                     xot(k)[:, t * TBL:(t + 1) * TBL],
                                     start=(k == 0), stop=(k == 7))
            for k in range(klo, khi):
                nc.tensor.matmul(tl[:], w1t(k, j * 128, (j + 1) * 128),
                                 xpt(k)[:, TC - 4:TC],
                                 start=(k == 0), stop=(k == 7))

    def evict_j(j, jps, tl):
        ka, kb = 2 * j - 1, 2 * j
        for t in range(2):
            nc.scalar.copy(
                pext[j][0:64, 4 + ka + t * TBL:4 + ka + (t + 1) * TBL],
                jps[t][0:64, :])
            nc.scalar.copy(
                pext[j][64:128, 4 + kb + t * TBL:4 + kb + (t + 1) * TBL],
                jps[t][64:128, :])
        nc.scalar.copy(pext[j][0:64, 4:4 + ka], tl[0:64, 4 - ka:4])
        nc.scalar.copy(pext[j][64:128, 4:4 + kb], tl[64:128, 4 - kb:4])

    j1ps = [pA.tile([128, TBL], DT, tag="pa", name=f"j1_ps{t}")
            for t in range(2)]
    tl1 = pC.tile([128, 4], DT, tag="pc", name="tl1")
    if CARRY == "pe":
        emit_iown_mm()
        emit_j(1, 0, 4, j1ps, tl1)
        emit_buo(0, 0)
        emit_buo(1, 0)
        emit_j(1, 4, 8, j1ps, tl1)
        evict_j(1, j1ps, tl1)
        emit_buo(2, 0)
        emit_buo(3, 0)
        j2ps = [pA.tile([128, TBL], DT, tag="pa", name=f"j2_ps{t}")
                for t in range(2)]
        tl2 = pC.tile([128, 4], DT, tag="pc", name="tl2")
        emit_j(2, 0, 4, j2ps, tl2)
        emit_buo(4, 0)
        emit_buo(5, 0)
        emit_j(2, 4, 8, j2ps, tl2)
        evict_j(2, j2ps, tl2)
        emit_buo(6, 0)
        emit_buo(7, 0)
    elif INTERLEAVE:
        emit_buo(0, 0)
        emit_buo(1, 0)
        emit_j(1, 0, 4, j1ps, tl1)
        emit_buo(2, 0)
        emit_buo(3, 0)
        emit_j(1, 4, 8, j1ps, tl1)
        evict_j(1, j1ps, tl1)
        emit_buo(4, 0)
        emit_buo(5, 0)
        j2ps = [pA.tile([128, TBL], DT, tag="pa", name=f"j2_ps{t}")
                for t in range(2)]
        tl2 = pC.tile([128, 4], DT, tag="pc", name="tl2")
        emit_j(2, 0, 4, j2ps, tl2)
        emit_buo(6, 0)
        emit_buo(7, 0)
        emit_j(2, 4, 8, j2ps, tl2)
        evict_j(2, j2ps, tl2)
    else:
        emit_j(1, 0, 8, j1ps, tl1)
        evict_j(1, j1ps, tl1)
        j2ps = [pA.tile([128, TBL], DT, tag="pa", name=f"j2_ps{t}")
                for t in range(2)]
        tl2 = pC.tile([128, 4], DT, tag="pc", name="tl2")
        emit_j(2, 0, 8, j2ps, tl2)
        evict_j(2, j2ps, tl2)
        for n in range(8):
            emit_buo(n, 0)

    for n in range(4):
        emit_buo(n, 1)

    # ---------------- CH1 + Y per t-block, streamed out ----------------------
    ysb = [[yp.tile([128, 4 * TBL], ODT, tag=f"y{tb}{g}", name=f"y{tb}{g}")
            for g in range(2)] for tb in range(2)]

    def emit_ch1(tb):
        cps = pC.tile([64, TBL], DT, tag="pc", name=f"c_ps{tb}")
        for n in range(8):
            nc.tensor.matmul(cps[:], c1sb[:, n * 64:(n + 1) * 64],
                             hsb[n][:, tb * TBL:(tb + 1) * TBL],
                             start=(n == 0), stop=(n == 7))
        nc.scalar.copy(pext[0][0:64, 4 + tb * TBL:4 + (tb + 1) * TBL], cps[:])

    def emit_y(tb):
        for o in range(8):
            yps = pA.tile([128, TBL], DT, tag="pa", name=f"y_ps{o}_{tb}")
            for m in range(3):
                nc.tensor.matmul(yps[:], w2t(m, o),
                                 pext[m][:, 4 + tb * TBL:4 + (tb + 1) * TBL],
                                 start=(m == 0), stop=(m == 2))
            g, oo = divmod(o, 4)
            dst = ysb[tb][g][:, oo * TBL:(oo + 1) * TBL]
            if (tb == 0 and o < 6) or not VECCOPY:
                nc.scalar.copy(dst, yps[:])
            else:
                nc.vector.tensor_copy(dst, yps[:])
            if oo == 3:
                nc.sync.dma_start(
                    yt[:, tb * 4096 + g * 2048:tb * 4096 + (g + 1) * 2048],
                    ysb[tb][g][:])

    if SPLIT_SCAN:
        emit_ch1(0)
        for n in range(4, 8):
            emit_buo(n, 1)
    else:
        for n in range(4, 8):
            emit_buo(n, 1)
        emit_ch1(0)
    emit_y(0)
    emit_ch1(1)
    emit_y(1)


def _build():
    nc = bacc.Bacc("TRN2", target_bir_lowering=False, debug=False,
                   num_devices=8)
    xo = nc.dram_tensor("xo", [128, 8192], MDT, kind="ExternalInput").ap()
    xp = nc.dram_tensor("xp", [128, 8192], MDT, kind="ExternalInput").ap()
    w1 = nc.dram_tensor("w1", [128, 3072], MDT, kind="ExternalInput").ap()
    b2 = nc.dram_tensor("b2", [64, 1024], MDT, kind="ExternalInput").ap()
    c1 = nc.dram_tensor("c1", [128, 512], MDT, kind="ExternalInput").ap()
    w2 = nc.dram_tensor("w2", [128, 3072], MDT, kind="ExternalInput").ap()
    if CARRY == "pe":
        ap = nc.dram_tensor("ap", [128, 1160], DT, kind="ExternalInput").ap()
    else:
        ap = nc.dram_tensor("ap", [128, 8192], MDT, kind="ExternalInput").ap()
    avio = nc.dram_tensor("avio", [128, 16], DT, kind="ExternalInput").ap()
    ident = nc.dram_tensor("ident", [64, 65], MDT, kind="ExternalInput").ap()
    yt = nc.dram_tensor("yt", [128, 8192], ODT, kind="ExternalOutput").ap()

    with tile.TileContext(nc) as tc, ExitStack() as ctx:
        _emit(ctx, tc, (xo, xp, w1, b2, c1, w2, ap, avio, ident, yt))
    nc.compile()
    return nc


def _get_nc():
    global _CACHED_NC
    if _CACHED_NC is None:
        _CACHED_NC = _build()
    return _CACHED_NC


def _pack_kt(arr):
    """[1024, C] -> [128, 8*C] with blocks of 128 rows side by side."""
    C = arr.shape[1]
    return np.ascontiguousarray(
        arr.reshape(8, 128, C).transpose(1, 0, 2).reshape(128, 8 * C))


def kernel(inputs, h0, A, B1, B2, C1, C2, M1, M2):
    global LAST_RESULT
    X = np.asarray(inputs, dtype=F32)
    h0 = np.asarray(h0, dtype=F32)
    A64 = np.asarray(A, dtype=np.float64)

    W1 = np.concatenate(
        [np.asarray(B1, dtype=F32)]
        + [np.ascontiguousarray(np.asarray(M1, dtype=F32)[:, :, k].T)
           for k in range(KX)], axis=1)
    W2 = np.concatenate(
        [np.asarray(C2, dtype=F32)]
        + [np.ascontiguousarray(np.asarray(M2, dtype=F32)[:, :, k].T)
           for k in range(KX)], axis=0)
    w1kt = W1.astype(MNP).reshape(8, 128, 384)           # [k, p, c]
    w1p = np.concatenate([                               # [128, 1024 | 2048]
        np.ascontiguousarray(w1kt[:, :, 0:128].transpose(1, 0, 2)
                             .reshape(128, 1024)),
        np.ascontiguousarray(w1kt[:, :, 128:384].transpose(1, 0, 2)
                             .reshape(128, 2048))], axis=1)
    w2p = np.ascontiguousarray(
        W2.reshape(3, 128, 1024).transpose(1, 0, 2)
        .reshape(128, 3072).astype(MNP))                 # [128, 3072]
    b2c = np.ascontiguousarray(np.asarray(B2, dtype=F32).astype(MNP))
    c1p = _pack_kt(np.asarray(C1, dtype=F32).astype(MNP))  # [128, 512]

    # decay powers AP[n, s] = A[n]^(1023-s); for "pe" the device builds them
    # from lnA + per-partition exponents (saves 2MB of DMA)
    if CARRY == "pe":
        app = np.zeros((128, 1160), F32)
        app[0, 0:1024] = np.log(A64).astype(F32)
        for st in range(8):
            app[:, 1024 + st] = (TC - 1) - 128 * st - np.arange(128)
        app[0, 1032:1160] = 1.0
        app = np.ascontiguousarray(app)
    else:
        powers = (TC - 1) - np.arange(TC, dtype=np.float64)
        APm = np.exp(np.log(A64)[:, None] * powers[None, :])  # [1024,1024]
        app = _pack_kt(APm.astype(F32).astype(MNP))           # [128, 8192]
    identm = np.zeros((64, 65), F32)
    identm[:, 0:64] = np.eye(64, dtype=F32)
    identm[:, 64] = 1.0
    identm = np.ascontiguousarray(identm.astype(MNP))

    ioff_h0 = h0                                           # half 0: plain h0
    ioff_h1 = (A64 ** TC * h0.astype(np.float64)).astype(F32)  # A^1024 h0

    Xbf = X.astype(MNP)
    zeros_xp = np.zeros((128, 8192), MNP)

    def pack_x(b, sl):
        return _pack_kt(np.ascontiguousarray(Xbf[b, sl, :].T))

    in_maps = []
    for c in range(8):
        b, half = divmod(c, 2)
        xoc = pack_x(b, slice(half * TC, (half + 1) * TC))
        if half == 0:
            xpc, ioff = zeros_xp, ioff_h0
        else:
            xpc, ioff = pack_x(b, slice(0, TC)), ioff_h1
        avio = np.zeros((128, 16), F32)
        avio[:, 0:8] = np.asarray(A, dtype=F32).reshape(8, 128).T
        avio[:, 8:16] = ioff.reshape(8, 128).T
        in_maps.append({"xo": xoc, "xp": xpc, "w1": w1p, "b2": b2c,
                        "c1": c1p, "w2": w2p, "ap": app, "avio": avio,
                        "ident": identm})

    nc = _get_nc()
    trace = bool(int(os.environ.get("KERNEL_TRACE", "0")))
    LAST_RESULT = run_bass_kernel_spmd(nc, in_maps, core_ids=list(range(8)),
                                       trace=trace)
    Y = np.empty((B, T, OUT), F32)
    for c in range(8):
        b, half = divmod(c, 2)
        ytc = np.asarray(LAST_RESULT.results[c]["yt"], dtype=F32)
        # yt[p, tb*4096 + o*512 + t] -> Y_core[o*128+p, tb*512+t]
        yc = ytc.reshape(128, 2, 8, 512).transpose(2, 0, 1, 3).reshape(1024, 1024)
        Y[b, half * TC:(half + 1) * TC, :] = yc.T
    return Y
